# revision 1
# baseline (speedup 1.0000x reference)
"""Multi-head self-attention (B=2, T=2048, D=1024, H=16) on 8 TRN2 NeuronCores.

Sharding: core c -> (b = c // 4, head-group hg = c % 4); each core computes the
full causal attention + partial output projection for its 4 heads of one batch
element.  The host pre-transposes x, pre-slices Wqkv columns / Wout rows per
head group, and sums the 4 bf16 partial projections per batch element (+ bout)
at the end.

Device-side dataflow (per core), all matmuls bf16.  Phases A and B share
one tile pool (PSUM re-tagged onto B's tags) so there is no inter-phase
barrier: B's first query tile's S-matmuls and exps overlap A's V section.
  A) qkT[c,t] = W[:,c].T @ xT   (c-major; heads packed 2-per-128-partitions;
     o-outer/it-inner loop so each LDWEIGHTS serves 4 matmuls).  Inputs arrive
     as a few large DMAs ordered so the first matmul chain starts immediately.
     V[t,c]   = xT[:,t].T @ Wv  (natural layout, split-ones augmented:
     even head lhsT = [V|1], odd head lhsT = [1|V], so each AV matmul emits
     ctx on one 64-row half and replicated softmax denominators on the other,
     with the ctx halves of a head pair landing on complementary halves)
  B) S^T[j,i] = kT.T @ qT  (two heads row-packed at rows 0:64 / 64:128 -> the
     two K=64 matmuls run concurrently via PE row tiling)
     P^T = exp(S^T / 8), split so neither activation engine gates the PE:
       - diagonal-class blocks (exactness matters for short rows): ScalarE
         ACT Exp on the live columns + small DVE triangle multiplies; the
         fully-masked column prefix stays zero via persistent pre-zeroed
         tiles.  No max-subtraction: scores ~N(0,1), fp32 cannot overflow.
       - sub-diagonal blocks: cross-engine per slot -- head-half 0 on ScalarE
         (exact), head-half 1 on VectorE via a ONE-op Schraudolph: i16 =
         convert(S*A/2^16 + B/2^16), whose bit pattern IS the bf16 of
         exp(S/8) (~3% sawtooth; consistent numerator/denominator and
         long-row averaging make the end-to-end output error match the
         exact path -- verified).  Every 5th slot runs both halves on DVE.
     ctx^T/sums accumulate in PSUM per head pair over j-blocks; the
     diagonal-class S and AV matmuls stream only the live columns
     i >= 128q (the masked prefix never enters the PE).  The
     emission is software-pipelined: slot n+1's score matmuls are emitted
     before slot n's exp, and the AV matmuls lag one further slot, so the
     PE always has work queued behind every exp wait and matmuls run
     back-to-back instead of at isolated-matmul rate.
     finish: full-width reciprocal_approx_fast straight off the PSUM tile,
     two small SBUF->SBUF DMAs cross the replicated-sums halves into rec_n,
     aligned unnormalized ctx drains, then a deferred GpSimd multiply
     normalizes merged in place (overlapped with the next query tile).
  C) out^T[e,t] = Wout_shard.T @ ctx^T (transposed so each e-block's weights
     stay loaded across four back-to-back t-chunk matmuls), bf16, one large
     contiguous DMA per e-block; the host transposes and accumulates.
"""

import math
from contextlib import ExitStack

import numpy as np
import ml_dtypes

import concourse.bass as bass
import concourse.bacc as bacc_mod
import concourse.mybir as mybir
import concourse.tile as tile
FP32 = mybir.dt.float32
INT16 = mybir.dt.int16
FP32R = mybir.dt.float32r
BF16 = mybir.dt.bfloat16
AF = mybir.ActivationFunctionType
ALU = mybir.AluOpType

B, T, D, H = 2, 2048, 1024, 16
Dh = D // H          # 64
NCORES = 8
HPC = 4              # heads per core
NPAIR = HPC // 2     # head pairs per core (2 heads share a 128-partition block)
IT = T // 512        # 4 query tiles of 512
JB = T // 128        # 16 key blocks of 128
KO = D // 128        # 8 contraction blocks for the projections
SCALE = 1.0 / math.sqrt(Dh)

# Schraudolph bit-trick exp(s/8): i32 = round(s*A + B), bitcast to fp32.
# ~3% sawtooth rel err; used only on sub-diagonal blocks where long-row
# averaging + consistent denominators make it indistinguishable from exact
# (verified: end-to-end output error matches the exact path).
SCH_A16 = float((1 << 23) * (1.4426950408889634 / 8.0) / 65536.0)
SCH_B16 = float((127 * (1 << 23) - 365000) / 65536.0)


def build_program(compile=True):
    nc = bacc_mod.Bacc()

    xT = nc.declare_dram_parameter("xT", [D, T], BF16, isOutput=False)
    wqk = nc.declare_dram_parameter("wqk", [128, KO, 2 * HPC * Dh], BF16,
                                    isOutput=False)
    wv = nc.declare_dram_parameter("wv", [128, KO, HPC * Dh], BF16,
                                   isOutput=False)
    wout = nc.declare_dram_parameter("wout", [128, 2, D], BF16, isOutput=False)
    tri = nc.declare_dram_parameter("tri", [128, 256], BF16, isOutput=False)
    out = nc.declare_dram_parameter("outT", [D, T], BF16, isOutput=True)

    xT_r = xT.rearrange("(o p) t -> p o t", p=128)

    with ExitStack() as ctx:
        tc = ctx.enter_context(tile.TileContext(nc))
        persist = ctx.enter_context(tc.tile_pool(name="persist", bufs=1))

        # ---------------- persistent tiles ----------------
        qkT = {}
        for nm in ("qT0", "qT1", "kT0", "kT1"):
            qkT[nm] = persist.tile([128, T], BF16, name=nm, tag=nm)
        V_aug = persist.tile([128, JB, HPC, 128], BF16, name="V_aug", tag="V_aug")
        merged = [
            persist.tile([128, IT, 512], BF16, name=f"merged{p}", tag=f"merged{p}")
            for p in range(NPAIR)
        ]
        wout_sb = persist.tile([128, 2, D], BF16, name="wout_sb", tag="wout_sb")
        tri_sb = persist.tile([128, 2, 128], BF16, name="tri_sb", tag="tri_sb")

        diag_pT = {
            (q, pr): persist.tile([128, 2, 512], BF16, name=f"pTd{q}_{pr}",
                                  tag=f"pTd{q}_{pr}")
            for q in range(4) for pr in range(NPAIR)
        }

        def load_consts():
            # split-ones: even heads [V|1], odd heads [1|V]
            nc.gpsimd.memset(V_aug[:, :, 0::2, 64:128], 1.0)
            nc.gpsimd.memset(V_aug[:, :, 1::2, 0:64], 1.0)
            # fully-masked column prefix [0, 128q) of diagonal P^T tiles
            for (q, pr), t_ in diag_pT.items():
                if q > 0:
                    nc.gpsimd.memset(t_[:, :, : 128 * q], 0.0)

        # ---------------- phases A+B: one pool, no barrier ----------------
        with (
            tc.tile_pool(name="phB", bufs=2) as pb,
            tc.tile_pool(name="psB", bufs=1, space="PSUM") as psb,
        ):
            xT_sb = pb.tile([128, KO, T], BF16, name="xT_sb", tag="xT_sb", bufs=1)
            wqk_sb = pb.tile([128, KO, 2 * HPC * Dh], BF16, name="wqk_sb",
                             tag="wqk_sb", bufs=1)
            wv_sb = pb.tile([128, KO, HPC * Dh], BF16, name="wv_sb", tag="wv_sb",
                            bufs=1)
            nc.sync.dma_start(wqk_sb[:, 0], wqk[:, 0])
            nc.sync.dma_start(xT_sb[:, 0, 0:512], xT_r[:, 0, 0:512])
            nc.sync.dma_start(xT_sb[:, 0, 512:], xT_r[:, 0, 512:])
            nc.sync.dma_start(wqk_sb[:, 1:], wqk[:, 1:])
            for o in range(1, KO, 2):
                o2 = min(o + 2, KO)
                nc.sync.dma_start(xT_sb[:, o:o2], xT_r[:, o:o2])
            nc.sync.dma_start(wv_sb[:], wv[:])
            nc.sync.dma_start(wout_sb[:], wout[:])
            nc.sync.dma_start(tri_sb[:], tri[:])
            load_consts()

            # qT/kT: [c, t] c-major (cb: 0,1 -> q pairs; 2,3 -> k pairs).
            dests = [qkT["qT0"], qkT["qT1"], qkT["kT0"], qkT["kT1"]]
            for cb in range(4):
                qk_t = [
                    psb.tile([128, 2, 512], FP32, name="ps_qk", tag="ps_s",
                             bufs=2),
                    psb.tile([128, 2, 512], FP32, name="ps_qk2",
                             tag=f"psum_ctx{cb % 2}", bufs=1),
                ]
                pss = [qk_t[i // 2][:, i % 2, :] for i in range(IT)]
                for o in range(KO):
                    for it in range(IT):
                        nc.tensor.matmul(
                            pss[it],
                            lhsT=wqk_sb[:, o, 128 * cb: 128 * (cb + 1)],
                            rhs=xT_sb[:, o, 512 * it: 512 * (it + 1)],
                            start=(o == 0), stop=(o == KO - 1),
                        )
                for it in range(IT):
                    if it % 2 == 0:
                        nc.scalar.copy(
                            dests[cb][:, 512 * it: 512 * (it + 1)], pss[it]
                        )
                    else:
                        nc.vector.tensor_copy(
                            dests[cb][:, 512 * it: 512 * (it + 1)], pss[it]
                        )

            # V natural [t, c]; drain per parity into the split-ones layout
            v_t = {}
            for tb in range(JB):
                if tb % 2 == 0:
                    v_t[tb // 2] = psb.tile(
                        [128, 2, 512], FP32, name="ps_v",
                        tag=f"psum_ctx{(tb // 2) % 2}", bufs=1)
                psv = v_t[tb // 2][:, tb % 2, 0:256]
                for o in range(KO):
                    nc.tensor.matmul(
                        psv,
                        lhsT=xT_sb[:, o, 128 * tb: 128 * (tb + 1)],
                        rhs=wv_sb[:, o],
                        start=(o == 0), stop=(o == KO - 1),
                    )
                psv_r = psv.rearrange("p (h d) -> p h d", h=HPC)
                deng = nc.scalar if tb % 3 == 2 else nc.vector
                if tb % 3 == 2:
                    nc.scalar.copy(V_aug[:, tb, 0::2, 0:64], psv_r[:, 0::2, :])
                    nc.scalar.copy(V_aug[:, tb, 1::2, 64:128], psv_r[:, 1::2, :])
                else:
                    nc.vector.tensor_copy(V_aug[:, tb, 0::2, 0:64],
                                          psv_r[:, 0::2, :])
                    nc.vector.tensor_copy(V_aug[:, tb, 1::2, 64:128],
                                          psv_r[:, 1::2, :])

            # ---------------- phase B: attention (same pool) ----------
            rec_n = [
                persist.tile([128, IT, 512], FP32, name=f"rec_n{p}",
                             tag=f"rec_n{p}")
                for p in range(NPAIR)
            ]

            def dve_exp(pT_ap, ps_ap):
                """exp(x/8) on VectorE in ONE native op: Schraudolph with the
                affine landing scaled by 2^-16 so the int16 convert keeps the
                top 16 bits of the fp32 pattern = the bf16 result directly."""
                nc.vector.tensor_scalar(
                    out=pT_ap.bitcast(INT16), in0=ps_ap,
                    scalar1=SCH_A16, scalar2=SCH_B16,
                    op0=ALU.mult, op1=ALU.add,
                )

            def finish_pair(it, pair, psum_ctx):
                """Unnormalized aligned ctx drains; full-width reciprocals of
                the replicated sums rows; two small SBUF->SBUF DMAs cross the
                reciprocal halves into rec_n; normalization is deferred to
                GpSimd (overlapped with the next it)."""
                recs = pb.tile([128, 2, 512], FP32, name="recs", tag="rec",
                               bufs=2)
                nc.vector.reciprocal_approx_fast(recs[:], psum_ctx[:])
                nc.sync.dma_start(rec_n[pair][0:64, it, :], recs[64:128, 0, :])
                nc.sync.dma_start(rec_n[pair][64:128, it, :], recs[0:64, 1, :])
                nc.scalar.copy(merged[pair][0:64, it], psum_ctx[0:64, 0, :])
                nc.vector.tensor_copy(merged[pair][64:128, it],
                                      psum_ctx[64:128, 1, :])
                nc.gpsimd.tensor_tensor(
                    out=merged[pair][:, it], in0=merged[pair][:, it],
                    in1=rec_n[pair][:, it, :], op=ALU.mult,
                )

            fullct = [0]
            for it in range(IT):
                isl = slice(512 * it, 512 * (it + 1))
                njb = 4 * it + 4  # causal: j blocks 0 .. 4it+3
                ctxs = [
                    psb.tile([128, 2, 512], FP32, name="psum_ctx",
                             tag=f"psum_ctx{pair}", bufs=1)
                    for pair in range(NPAIR)
                ]
                # software-pipelined by one slot: emit slot n+1's score
                # matmuls before slot n's exp/AV so the PE always has an
                # S-pair queued behind the exp wait (keeps matmuls
                # back-to-back instead of isolated-rate)
                slots = [(jb, pair) for jb in range(njb)
                         for pair in range(NPAIR)]
                ps2s = {}

                def emit_S(n):
                    jb, pair = slots[n]
                    jsl = slice(128 * jb, 128 * (jb + 1))
                    c0 = max(0, 128 * (jb - 4 * it))
                    isl_t = slice(512 * it + c0, 512 * (it + 1))
                    ps2 = psb.tile([128, 2, 512], FP32, name="ps_s",
                                   tag="ps_s", bufs=2)
                    for hl in range(2):
                        rows = slice(64 * hl, 64 * (hl + 1))
                        nc.tensor.matmul(
                            ps2[:, hl, c0:],
                            lhsT=qkT[f"kT{pair}"][rows, jsl],
                            rhs=qkT[f"qT{pair}"][rows, isl_t],
                            start=True, stop=True,
                        )
                    ps2s[n] = ps2

                emit_S(0)
                pending = [None]

                def emit_AV(jb_, pair_, pT_):
                    c0 = max(0, 128 * (jb_ - 4 * it))
                    for hl in range(2):
                        h = 2 * pair_ + hl
                        nc.tensor.matmul(
                            ctxs[pair_][:, hl, c0:],
                            lhsT=V_aug[:, jb_, h, :],
                            rhs=pT_[:, hl, c0:],
                            start=(jb_ == 0), stop=(jb_ == njb - 1),
                        )

                for n, (jb, pair) in enumerate(slots):
                    if n + 1 < len(slots):
                        emit_S(n + 1)
                    q = jb - 4 * it
                    ps2 = ps2s.pop(n)
                    if q < 0:  # fully sub-diagonal block
                        pT = pb.tile([128, 2, 512], BF16, name="pT",
                                     tag="pT_full", bufs=4)
                        fi = fullct[0]; fullct[0] += 1
                        if fi % 5 == 4:    # whole slot on DVE
                            dve_exp(pT[:], ps2[:])
                        else:              # cross-engine: hl0 Sca, hl1 Vec
                            nc.scalar.activation(pT[:, 0, :], ps2[:, 0, :],
                                                 AF.Exp, scale=SCALE)
                            dve_exp(pT[:, 1, :], ps2[:, 1, :])
                    else:      # diagonal-class block: ScalarE exp + DVE tri
                        pT = diag_pT[(q, pair)]
                        nc.scalar.activation(
                            pT[:, :, 128 * q:], ps2[:, :, 128 * q:],
                            AF.Exp, scale=SCALE,
                        )
                        for hl in range(2):
                            tri_eng = nc.gpsimd if hl == 0 else nc.vector
                            tri_eng.tensor_tensor(
                                out=pT[:, hl, 128 * q: 128 * (q + 1)],
                                in0=pT[:, hl, 128 * q: 128 * (q + 1)],
                                in1=tri_sb[:, hl],
                                op=ALU.mult,
                            )
                    # AV lags one slot: emit the previous slot's AV now
                    # (its exp has had a full slot to finish)
                    if pending[0] is not None:
                        emit_AV(*pending[0])
                    pending[0] = (jb, pair, pT)
                emit_AV(*pending[0])
                for pair in range(NPAIR):
                    finish_pair(it, pair, ctxs[pair])


        # ---------------- phase C: output projection (transposed) --------
        # out^T[e, t] = Wout_shard^T @ ctx^T: lhsT = wout e-block stays loaded
        # across the four t-chunk matmuls -> no LDWEIGHTS between matmuls.
        with (
            tc.tile_pool(name="phC", bufs=4) as pc_,
            tc.tile_pool(name="psC", bufs=2, space="PSUM") as psc,
        ):
            merged_flat = [m.rearrange("p a b -> p (a b)") for m in merged]
            for eb in range(D // 128):
                psos = [
                    psc.tile([128, 512], FP32, name="ps_o", tag="ps_o", bufs=8)
                    for _ in range(4)
                ]
                for pair in range(NPAIR):
                    wsl = wout_sb[:, pair, 128 * eb: 128 * (eb + 1)]
                    for tc_ in range(4):
                        nc.tensor.matmul(
                            psos[tc_][:],
                            lhsT=wsl,
                            rhs=merged_flat[pair][:, 512 * tc_: 512 * (tc_ + 1)],
                            start=(pair == 0), stop=(pair == NPAIR - 1),
                        )
                osb = pc_.tile([128, T], BF16, name="osb", tag="osb",
                               bufs=2)
                for tc_ in range(4):
                    if tc_ % 2 == 0:
                        nc.scalar.copy(osb[:, 512 * tc_: 512 * (tc_ + 1)],
                                       psos[tc_][:])
                    else:
                        nc.vector.tensor_copy(
                            osb[:, 512 * tc_: 512 * (tc_ + 1)], psos[tc_][:])
                nc.sync.dma_start(out[128 * eb: 128 * (eb + 1), :], osb[:])

    if compile:
        nc.compile()
    return nc


_PROGRAM = None


def _get_program():
    global _PROGRAM
    if _PROGRAM is None:
        _PROGRAM = build_program()
    return _PROGRAM


def _tri():
    dj = np.arange(128)[:, None]
    di = np.arange(128)[None, :]
    t = (dj <= di).astype(ml_dtypes.bfloat16)
    return np.ascontiguousarray(np.concatenate([t, t], axis=1))


def make_in_maps(x, Wqkv, Wout):
    in_maps = []
    for core in range(NCORES):
        b, hg = core // (NCORES // B), core % (NCORES // B)
        c0 = hg * HPC * Dh
        csl = slice(c0, c0 + HPC * Dh)
        wqk_full = np.concatenate(
            [Wqkv[:, csl], Wqkv[:, D + c0: D + c0 + HPC * Dh]], axis=1
        ).astype(ml_dtypes.bfloat16)
        wv_full = Wqkv[:, 2 * D + c0: 2 * D + c0 + HPC * Dh].astype(
            ml_dtypes.bfloat16)
        in_maps.append({
            "tri": _tri(),
            "xT": np.ascontiguousarray(x[b].T).astype(ml_dtypes.bfloat16),
            "wqk": np.ascontiguousarray(
                wqk_full.reshape(KO, 128, 2 * HPC * Dh).transpose(1, 0, 2)),
            "wv": np.ascontiguousarray(
                wv_full.reshape(KO, 128, HPC * Dh).transpose(1, 0, 2)),
            "wout": np.ascontiguousarray(
                Wout[csl, :].astype(ml_dtypes.bfloat16)
                .reshape(2, 128, D).transpose(1, 0, 2)),
        })
    return in_maps


def kernel(x, causal_mask, key_padding_mask, Wqkv, bqkv, Wout, bout,
           _trace=False):
    from concourse.bass_utils import run_bass_kernel_spmd

    x = np.asarray(x, dtype=np.float32)
    Wqkv = np.asarray(Wqkv, dtype=np.float32)
    Wout = np.asarray(Wout, dtype=np.float32)
    bqkv = np.asarray(bqkv, dtype=np.float32)
    bout = np.asarray(bout, dtype=np.float32)
    if np.any(np.asarray(key_padding_mask)):
        raise NotImplementedError("key_padding_mask with padded keys")
    if np.any(bqkv):
        raise NotImplementedError("nonzero bqkv")

    nc = _get_program()
    in_maps = make_in_maps(x, Wqkv, Wout)
    res = run_bass_kernel_spmd(nc, in_maps, core_ids=list(range(NCORES)),
                               trace=_trace)
    G = NCORES // B
    outp = np.empty((B, T, D), dtype=np.float32)
    for b in range(B):
        acc = res.results[b * G]["outT"].astype(np.float32)
        for hg in range(1, G):
            acc += res.results[b * G + hg]["outT"].astype(np.float32)
        outp[b] = acc.T + bout
    kernel.last_exec_time_ns = res.exec_time_ns
    return outp



# revision 2
# speedup vs baseline: 1.0002x; 1.0002x over previous
"""Multi-head self-attention (B=2, T=2048, D=1024, H=16) on 8 TRN2 NeuronCores.

Sharding: core c -> (b = c // 4, head-group hg = c % 4); each core computes the
full causal attention + partial output projection for its 4 heads of one batch
element.  The host pre-transposes x, pre-slices Wqkv columns / Wout rows per
head group, and sums the 4 bf16 partial projections per batch element (+ bout)
at the end.

v2 schedule (single TileContext, everything software-pipelined):
  - 8 warmup matmuls on a memset tile pre-warm the PE HAM clock gate during
    the initial DMA window.
  - q/k projection runs o-OUTER over T-quarters so each quarter only needs a
    prefix of the xT DMA stream (DMA stays ahead of the PE; no stalls).
  - S^T matmuls + exp run DECOUPLED ahead of the AV stream (run-ahead cursor):
    the exp work (the real bottleneck engine-wise) starts while the PE is
    still busy with projections.  Sub-diagonal blocks: ScalarE exact exp on
    head-half 0, VectorE one-op Schraudolph on head-half 1.  Diagonal blocks:
    exact ScalarE exp on live columns + one fused [2,128] triangle multiply.
  - AV accumulates per (query-tile, head-pair) with pair-OUTER psum rotation
    over two explicit rings, so a pair's softmax-finish (reciprocal + two
    half-crossing DMAs + psum*rec -> bf16 merged) overlaps the next pair's
    compute; no query-tile-boundary stalls.
  - The output projection (phase C) is interleaved per query-tile INTO the
    attention stream (emitted during the next tile's AV slots), with the
    out-DMA per (2 e-blocks, 512 tokens) chunk, so there is no serial tail.
  - All PSUM-drain copies are distributed over ScalarE/VectorE (GpSimd for
    SBUF-only ops) by a greedy load-balance model.
"""

import math
from contextlib import ExitStack

import numpy as np
import ml_dtypes

import concourse.bass as bass
import concourse.bacc as bacc_mod
import concourse.mybir as mybir
import concourse.tile as tile
FP32 = mybir.dt.float32
INT16 = mybir.dt.int16
BF16 = mybir.dt.bfloat16
AF = mybir.ActivationFunctionType
ALU = mybir.AluOpType

B, T, D, H = 2, 2048, 1024, 16
Dh = D // H          # 64
NCORES = 8
HPC = 4              # heads per core
NPAIR = HPC // 2     # head pairs per core (2 heads share a 128-partition block)
IT = T // 512        # 4 query tiles of 512
JB = T // 128        # 16 key blocks of 128
KO = D // 128        # 8 contraction blocks for the projections
SCALE = 1.0 / math.sqrt(Dh)

# Schraudolph bit-trick exp(s/8): i16 = round(s*A/2^16 + B/2^16), whose bit
# pattern IS the bf16 of exp(s/8) (~3% sawtooth).  Used only on sub-diagonal
# blocks where long-row averaging + consistent denominators make it
# indistinguishable from exact (verified end-to-end).
SCH_A16 = float((1 << 23) * (1.4426950408889634 / 8.0) / 65536.0)
SCH_B16 = float((127 * (1 << 23) - 365000) / 65536.0)


def build_program(compile=True):
    nc = bacc_mod.Bacc()

    xT = nc.declare_dram_parameter("xT", [D, T], BF16, isOutput=False)
    wqk = nc.declare_dram_parameter("wqk", [128, KO, 2 * HPC * Dh], BF16,
                                    isOutput=False)
    wv = nc.declare_dram_parameter("wv", [128, KO, HPC * Dh], BF16,
                                   isOutput=False)
    wout = nc.declare_dram_parameter("wout", [128, 2, D], BF16, isOutput=False)
    tri = nc.declare_dram_parameter("tri", [128, 256], BF16, isOutput=False)
    out = nc.declare_dram_parameter("outT", [D, T], BF16, isOutput=True)

    xT_r = xT.rearrange("(o p) t -> p o t", p=128)
    out_r = out.rearrange("(e p) t -> p e t", p=128)

    with ExitStack() as ctx:
        tc = ctx.enter_context(tile.TileContext(nc))
        persist = ctx.enter_context(tc.tile_pool(name="persist", bufs=1))
        pb = ctx.enter_context(tc.tile_pool(name="work", bufs=2))
        psb = ctx.enter_context(tc.tile_pool(name="ps", bufs=1, space="PSUM"))

        # ---------------- persistent tiles ----------------
        qkT = {}
        for nm in ("qT0", "qT1", "kT0", "kT1"):
            qkT[nm] = persist.tile([128, T], BF16, name=nm, tag=nm)
        V_aug = persist.tile([128, JB, HPC, 128], BF16, name="V_aug",
                             tag="V_aug")
        merged = [
            persist.tile([128, IT, 512], BF16, name=f"merged{p}",
                         tag=f"merged{p}")
            for p in range(NPAIR)
        ]
        rec_n = [
            persist.tile([128, IT, 512], FP32, name=f"rec_n{p}",
                         tag=f"rec_n{p}")
            for p in range(NPAIR)
        ]
        wout_sb = persist.tile([128, 2, D], BF16, name="wout_sb", tag="wout_sb")
        tri_sb = persist.tile([128, 2, 128], BF16, name="tri_sb", tag="tri_sb")
        warm = persist.tile([128, 512], BF16, name="warm", tag="warm")
        # diagonal-class P^T tiles, double-buffered by it parity
        diag_pT = {
            (db, q, pr): persist.tile([128, 2, 512], BF16,
                                      name=f"pTd{db}_{q}_{pr}",
                                      tag=f"pTd{db}_{q}_{pr}")
            for db in range(2) for q in range(4) for pr in range(NPAIR)
        }

        xT_sb = pb.tile([128, KO, T], BF16, name="xT_sb", tag="xT_sb", bufs=1)
        wqk_sb = pb.tile([128, KO, 2 * HPC * Dh], BF16, name="wqk_sb",
                         tag="wqk_sb", bufs=1)
        wv_sb = pb.tile([128, KO, HPC * Dh], BF16, name="wv_sb", tag="wv_sb",
                        bufs=1)

        # ---------------- engine load balancer ----------------
        loads = {"s": 0.0, "v": 0.0, "g": 0.0}

        def cost(eng, elems):
            if eng == "s":
                return 90 + 1.25 * elems
            if eng == "v":
                return 150 + 1.04 * elems
            return 340 + 1.6 * elems

        def drain(dst, src, elems):
            """PSUM->SBUF copy on the lighter of ScalarE/VectorE."""
            if loads["s"] + cost("s", elems) <= loads["v"] + cost("v", elems):
                loads["s"] += cost("s", elems)
                nc.scalar.copy(dst, src)
            else:
                loads["v"] += cost("v", elems)
                nc.vector.tensor_copy(dst, src)

        # ---------------- memsets ----------------
        nc.gpsimd.memset(warm[:], 0.0)
        # split-ones: even heads [V|1], odd heads [1|V]
        nc.gpsimd.memset(V_aug[:, :, 0::2, 64:128], 1.0)
        nc.gpsimd.memset(V_aug[:, :, 1::2, 0:64], 1.0)
        for (db, q, pr), t_ in diag_pT.items():
            if q > 0:
                nc.gpsimd.memset(t_[:, :, : 128 * q], 0.0)

        # ---------------- DMAs (ordered to match consumption) ----------------
        nc.sync.dma_start(wqk_sb[:, 0], wqk[:, 0])
        nc.sync.dma_start(xT_sb[:, 0, 0:512], xT_r[:, 0, 0:512])
        nc.sync.dma_start(wqk_sb[:, 1:], wqk[:, 1:])
        nc.sync.dma_start(xT_sb[:, 1, 0:512], xT_r[:, 1, 0:512])
        for o in range(2, KO, 2):
            nc.sync.dma_start(xT_sb[:, o:o + 2, 0:512],
                              xT_r[:, o:o + 2, 0:512])
        for q in range(1, 4):
            qsl = slice(512 * q, 512 * (q + 1))
            for o in range(0, KO, 2):
                nc.sync.dma_start(xT_sb[:, o:o + 2, qsl],
                                  xT_r[:, o:o + 2, qsl])
        nc.sync.dma_start(tri_sb[:], tri[:])
        nc.sync.dma_start(wv_sb[:], wv[:])
        nc.sync.dma_start(wout_sb[:], wout[:])

        # ---------------- PSUM rings ----------------
        # ring A/B: warmup, qk quarters 0&2, S slots, C groups
        # ring C/D: qk quarters 1&3, V tiles, AV ctx accumulators
        nAB = [0]
        nCD = [0]

        def tileAB(name):
            t = psb.tile([128, 2, 512], FP32, name=name,
                         tag=f"ps{'AB'[nAB[0] % 2]}", bufs=1)
            nAB[0] += 1
            return t

        def tileCD(name):
            t = psb.tile([128, 2, 512], FP32, name=name,
                         tag=f"ps{'CD'[nCD[0] % 2]}", bufs=1)
            nCD[0] += 1
            return t

        # ---------------- warmup (HAM pre-warm during DMA head) -------------
        wps = tileAB("warm_ps")
        for _ in range(8):
            nc.tensor.matmul(wps[:, 0], lhsT=warm[:, 0:128], rhs=warm[:],
                             start=True, stop=True)

        # ---------------- q/k projection: o-outer over T-quarters -----------
        dests = [qkT["qT0"], qkT["qT1"], qkT["kT0"], qkT["kT1"]]
        for q in range(4):
            qsl = slice(512 * q, 512 * (q + 1))
            ta = tileAB(f"qk{q}a") if q % 2 == 0 else tileCD(f"qk{q}a")
            tb_ = tileAB(f"qk{q}b") if q % 2 == 0 else tileCD(f"qk{q}b")
            for o in range(KO):
                for cb in range(4):
                    t_ = ta if cb < 2 else tb_
                    nc.tensor.matmul(
                        t_[:, cb % 2],
                        lhsT=wqk_sb[:, o, 128 * cb: 128 * (cb + 1)],
                        rhs=xT_sb[:, o, qsl],
                        start=(o == 0), stop=(o == KO - 1),
                    )
            for cb in range(4):
                t_ = ta if cb < 2 else tb_
                drain(dests[cb][:, qsl], t_[:, cb % 2], 512)

        # ---------------- S/exp run-ahead emitter ----------------
        slots = [(it, pair, jb)
                 for it in range(IT)
                 for pair in range(NPAIR)
                 for jb in range(4 * it + 4)]
        pTs = {}
        s_cur = [0]

        def dve_exp(pT_ap, ps_ap):
            nc.vector.tensor_scalar(
                out=pT_ap.bitcast(INT16), in0=ps_ap,
                scalar1=SCH_A16, scalar2=SCH_B16,
                op0=ALU.mult, op1=ALU.add,
            )

        def emit_S(n):
            it, pair, jb = slots[n]
            qv = jb - 4 * it
            jsl = slice(128 * jb, 128 * (jb + 1))
            ps2 = tileAB(f"s{n}")
            if qv < 0:
                c0 = 0
                pT = pb.tile([128, 2, 512], BF16, name="pT", tag="pT_full",
                             bufs=10)
            else:
                c0 = 128 * qv
                pT = diag_pT[(it % 2, qv, pair)]
            isl = slice(512 * it + c0, 512 * (it + 1))
            for hl in range(2):
                rows = slice(64 * hl, 64 * (hl + 1))
                nc.tensor.matmul(
                    ps2[:, hl, c0:],
                    lhsT=qkT[f"kT{pair}"][rows, jsl],
                    rhs=qkT[f"qT{pair}"][rows, isl],
                    start=True, stop=True,
                )
            if qv < 0:
                nc.scalar.activation(pT[:, 0, :], ps2[:, 0, :], AF.Exp,
                                     scale=SCALE)
                loads["s"] += cost("s", 512)
                dve_exp(pT[:, 1, :], ps2[:, 1, :])
                loads["v"] += cost("v", 512)
            else:
                nc.scalar.activation(pT[:, :, c0:], ps2[:, :, c0:], AF.Exp,
                                     scale=SCALE)
                loads["s"] += cost("s", 2 * (512 - c0))
                # fused triangle multiply over both head-halves
                el = 256
                if loads["g"] + cost("g", el) <= loads["v"] + cost("v", el):
                    loads["g"] += cost("g", el)
                    eng = nc.gpsimd
                else:
                    loads["v"] += cost("v", el)
                    eng = nc.vector
                eng.tensor_tensor(
                    out=pT[:, :, c0:c0 + 128], in0=pT[:, :, c0:c0 + 128],
                    in1=tri_sb[:], op=ALU.mult,
                )
            pTs[n] = pT

        def top_up(n):
            while s_cur[0] < min(n, len(slots)):
                emit_S(s_cur[0])
                s_cur[0] += 1

        # ---------------- V projection (4 packed psum tiles) -----------------
        # interleave S/exp pre-run for it0/it1 between tb groups
        pre = {0: 4, 1: 8, 2: 14, 3: 20}
        for vt in range(4):
            vtile = tileCD(f"v{vt}")
            for tb in range(4 * vt, 4 * vt + 4):
                psv = vtile[:, (tb % 4) // 2,
                            256 * (tb % 2): 256 * (tb % 2) + 256]
                for o in range(KO):
                    nc.tensor.matmul(
                        psv,
                        lhsT=xT_sb[:, o, 128 * tb: 128 * (tb + 1)],
                        rhs=wv_sb[:, o],
                        start=(o == 0), stop=(o == KO - 1),
                    )
                psv_r = psv.rearrange("p (h d) -> p h d", h=HPC)
                drain(V_aug[:, tb, 0::2, 0:64], psv_r[:, 0::2, :], 128)
                drain(V_aug[:, tb, 1::2, 64:128], psv_r[:, 1::2, :], 128)
            top_up(pre[vt])

        # ---------------- AV + finish + interleaved C ----------------
        def emit_AV(it, pair, jb, pT, ctx_t):
            njb = 4 * it + 4
            c0 = max(0, 128 * (jb - 4 * it))
            for hl in range(2):
                h = 2 * pair + hl
                nc.tensor.matmul(
                    ctx_t[:, hl, c0:],
                    lhsT=V_aug[:, jb, h, :],
                    rhs=pT[:, hl, c0:],
                    start=(jb == 0), stop=(jb == njb - 1),
                )

        def finish(it, pair, ctx_t):
            recs = pb.tile([128, 2, 512], FP32, name="recs", tag="recs",
                           bufs=2)
            nc.vector.reciprocal_approx_fast(recs[:], ctx_t[:])
            loads["v"] += cost("v", 1024)
            nc.sync.dma_start(rec_n[pair][0:64, it], recs[64:128, 0, :])
            nc.sync.dma_start(rec_n[pair][64:128, it], recs[0:64, 1, :])
            # merged = ctx * rec; two partition-halves, balanced per half:
            # (a) one VectorE TT straight off PSUM, or (b) ScalarE copy +
            #     GpSimd SBUF multiply.
            for half in range(2):
                rows = slice(64 * half, 64 * (half + 1))
                m_ = merged[pair][rows, it]
                c_ = ctx_t[rows, half, :]
                r_ = rec_n[pair][rows, it, :]
                ca = loads["v"] + cost("v", 512)
                cb_ = max(loads["s"] + cost("s", 512),
                          loads["g"] + cost("g", 512))
                if ca <= cb_:
                    loads["v"] += cost("v", 512)
                    nc.vector.tensor_tensor(out=m_, in0=c_, in1=r_,
                                            op=ALU.mult)
                else:
                    loads["s"] += cost("s", 512)
                    loads["g"] += cost("g", 512)
                    nc.scalar.copy(m_, c_)
                    nc.gpsimd.tensor_tensor(out=m_, in0=m_, in1=r_,
                                            op=ALU.mult)

        def emit_C_group(it_, gi):
            tpc = tileAB(f"c{it_}_{gi}")
            for pair in range(NPAIR):
                for e2 in range(2):
                    eb = 2 * gi + e2
                    nc.tensor.matmul(
                        tpc[:, e2],
                        lhsT=wout_sb[:, pair, 128 * eb: 128 * (eb + 1)],
                        rhs=merged[pair][:, it_],
                        start=(pair == 0), stop=(pair == NPAIR - 1),
                    )
            osb = pb.tile([128, 2, 512], BF16, name="osb", tag="osb", bufs=3)
            for e2 in range(2):
                drain(osb[:, e2], tpc[:, e2], 512)
            nc.sync.dma_start(
                out_r[:, 2 * gi: 2 * gi + 2, 512 * it_: 512 * (it_ + 1)],
                osb[:],
            )

        L = 6
        av_n = [0]
        cq = []
        for it in range(IT):
            njb = 4 * it + 4
            for pair in range(NPAIR):
                ctx_t = tileCD(f"ctx{it}_{pair}")
                for jb in range(njb):
                    n = av_n[0]
                    top_up(n + 1 + L)
                    if cq and jb % 3 == 2:
                        emit_C_group(*cq.pop(0))
                    emit_AV(it, pair, jb, pTs.pop(n), ctx_t)
                    av_n[0] += 1
                finish(it, pair, ctx_t)
                if pair == NPAIR - 1:
                    for gi in range(4):
                        cq.append((it, gi))
        while cq:
            emit_C_group(*cq.pop(0))

    if compile:
        nc.compile()
    return nc


_PROGRAM = None


def _get_program():
    global _PROGRAM
    if _PROGRAM is None:
        _PROGRAM = build_program()
    return _PROGRAM


def _tri():
    dj = np.arange(128)[:, None]
    di = np.arange(128)[None, :]
    t = (dj <= di).astype(ml_dtypes.bfloat16)
    return np.ascontiguousarray(np.concatenate([t, t], axis=1))


def make_in_maps(x, Wqkv, Wout):
    in_maps = []
    for core in range(NCORES):
        b, hg = core // (NCORES // B), core % (NCORES // B)
        c0 = hg * HPC * Dh
        csl = slice(c0, c0 + HPC * Dh)
        wqk_full = np.concatenate(
            [Wqkv[:, csl], Wqkv[:, D + c0: D + c0 + HPC * Dh]], axis=1
        ).astype(ml_dtypes.bfloat16)
        wv_full = Wqkv[:, 2 * D + c0: 2 * D + c0 + HPC * Dh].astype(
            ml_dtypes.bfloat16)
        in_maps.append({
            "tri": _tri(),
            "xT": np.ascontiguousarray(x[b].T).astype(ml_dtypes.bfloat16),
            "wqk": np.ascontiguousarray(
                wqk_full.reshape(KO, 128, 2 * HPC * Dh).transpose(1, 0, 2)),
            "wv": np.ascontiguousarray(
                wv_full.reshape(KO, 128, HPC * Dh).transpose(1, 0, 2)),
            "wout": np.ascontiguousarray(
                Wout[csl, :].astype(ml_dtypes.bfloat16)
                .reshape(2, 128, D).transpose(1, 0, 2)),
        })
    return in_maps


def kernel(x, causal_mask, key_padding_mask, Wqkv, bqkv, Wout, bout,
           _trace=False):
    from concourse.bass_utils import run_bass_kernel_spmd

    x = np.asarray(x, dtype=np.float32)
    Wqkv = np.asarray(Wqkv, dtype=np.float32)
    Wout = np.asarray(Wout, dtype=np.float32)
    bqkv = np.asarray(bqkv, dtype=np.float32)
    bout = np.asarray(bout, dtype=np.float32)
    if np.any(np.asarray(key_padding_mask)):
        raise NotImplementedError("key_padding_mask with padded keys")
    if np.any(bqkv):
        raise NotImplementedError("nonzero bqkv")

    nc = _get_program()
    in_maps = make_in_maps(x, Wqkv, Wout)
    res = run_bass_kernel_spmd(nc, in_maps, core_ids=list(range(NCORES)),
                               trace=_trace)
    G = NCORES // B
    outp = np.empty((B, T, D), dtype=np.float32)
    for b in range(B):
        acc = res.results[b * G]["outT"].astype(np.float32)
        for hg in range(1, G):
            acc += res.results[b * G + hg]["outT"].astype(np.float32)
        outp[b] = acc.T + bout
    kernel.last_exec_time_ns = res.exec_time_ns
    return outp


# revision 13
# speedup vs baseline: 1.0601x; 1.0599x over previous
"""Multi-head self-attention (B=2, T=2048, D=1024, H=16) on 8 TRN2 NeuronCores.

Sharding: core c -> (b = c // 4, head-group hg = c % 4); each core computes the
full causal attention + partial output projection for its 4 heads of one batch
element.  The host pre-transposes x, pre-slices Wqkv columns / Wout rows per
head group, and sums the 4 bf16 partial projections per batch element (+ bout)
at the end.

v2 schedule (single TileContext, everything software-pipelined):
  - 8 warmup matmuls on a memset tile pre-warm the PE HAM clock gate during
    the initial DMA window.
  - q/k projection runs o-OUTER over T-quarters so each quarter only needs a
    prefix of the xT DMA stream (DMA stays ahead of the PE; no stalls).
  - S^T matmuls + exp run DECOUPLED ahead of the AV stream (run-ahead cursor):
    the exp work (the real bottleneck engine-wise) starts while the PE is
    still busy with projections.  Sub-diagonal blocks: ScalarE exact exp on
    head-half 0, VectorE one-op Schraudolph on head-half 1.  Diagonal blocks:
    exact ScalarE exp on live columns + one fused [2,128] triangle multiply.
  - AV accumulates per (query-tile, head-pair) with pair-OUTER psum rotation
    over two explicit rings, so a pair's softmax-finish (reciprocal + two
    half-crossing DMAs + psum*rec -> bf16 merged) overlaps the next pair's
    compute; no query-tile-boundary stalls.
  - The output projection (phase C) is interleaved per query-tile INTO the
    attention stream (emitted during the next tile's AV slots), with the
    out-DMA per (2 e-blocks, 512 tokens) chunk, so there is no serial tail.
  - All PSUM-drain copies are distributed over ScalarE/VectorE (GpSimd for
    SBUF-only ops) by a greedy load-balance model.
"""

import math
from contextlib import ExitStack

import numpy as np
import ml_dtypes

import concourse.bass as bass
import concourse.bacc as bacc_mod
import concourse.mybir as mybir
import concourse.tile as tile
FP32 = mybir.dt.float32
INT16 = mybir.dt.int16
BF16 = mybir.dt.bfloat16
AF = mybir.ActivationFunctionType
ALU = mybir.AluOpType

B, T, D, H = 2, 2048, 1024, 16
Dh = D // H          # 64
NCORES = 8
HPC = 4              # heads per core
NPAIR = HPC // 2     # head pairs per core (2 heads share a 128-partition block)
IT = T // 512        # 4 query tiles of 512
JB = T // 128        # 16 key blocks of 128
KO = D // 128        # 8 contraction blocks for the projections
SCALE = 1.0 / math.sqrt(Dh)

# Schraudolph bit-trick exp(s/8): i16 = round(s*A/2^16 + B/2^16), whose bit
# pattern IS the bf16 of exp(s/8) (~3% sawtooth).  Used only on sub-diagonal
# blocks where long-row averaging + consistent denominators make it
# indistinguishable from exact (verified end-to-end).
SCH_A16 = float((1 << 23) * (1.4426950408889634 / 8.0) / 65536.0)
SCH_B16 = float((127 * (1 << 23) - 365000) / 65536.0)


def build_program(compile=True):
    nc = bacc_mod.Bacc()

    xT = nc.declare_dram_parameter("xT", [D, T], BF16, isOutput=False)
    wqk = nc.declare_dram_parameter("wqk", [128, KO, 2 * HPC * Dh], BF16,
                                    isOutput=False)
    wv = nc.declare_dram_parameter("wv", [128, KO, HPC * Dh], BF16,
                                   isOutput=False)
    wout = nc.declare_dram_parameter("wout", [128, 2, D], BF16, isOutput=False)
    tri = nc.declare_dram_parameter("tri", [128, 256], BF16, isOutput=False)
    out = nc.declare_dram_parameter("outT", [D, T], BF16, isOutput=True)

    xT_r = xT.rearrange("(o p) t -> p o t", p=128)
    out_r = out.rearrange("(e p) t -> p e t", p=128)

    with ExitStack() as ctx:
        tc = ctx.enter_context(tile.TileContext(nc))
        persist = ctx.enter_context(tc.tile_pool(name="persist", bufs=1))
        pb = ctx.enter_context(tc.tile_pool(name="work", bufs=2))
        psb = ctx.enter_context(tc.tile_pool(name="ps", bufs=1, space="PSUM"))

        # ---------------- persistent tiles ----------------
        qkT = {}
        for nm in ("qT0", "qT1", "kT0", "kT1"):
            qkT[nm] = persist.tile([128, T], BF16, name=nm, tag=nm)
        V_aug = persist.tile([128, JB, HPC, 128], BF16, name="V_aug",
                             tag="V_aug")
        merged = [
            persist.tile([128, IT, 512], BF16, name=f"merged{p}",
                         tag=f"merged{p}")
            for p in range(NPAIR)
        ]
        wout_sb = persist.tile([128, 2, D], BF16, name="wout_sb", tag="wout_sb")
        tri_sb = persist.tile([128, 2, 128], BF16, name="tri_sb", tag="tri_sb")
        warm = persist.tile([128, 512], BF16, name="warm", tag="warm")
        # diagonal-class P^T tiles, double-buffered by it parity
        diag_pT = {
            (db, q, pr): persist.tile([128, 2, 512], BF16,
                                      name=f"pTd{db}_{q}_{pr}",
                                      tag=f"pTd{db}_{q}_{pr}")
            for db in range(2) for q in range(4) for pr in range(NPAIR)
        }

        xT_sb = pb.tile([128, KO, T], BF16, name="xT_sb", tag="xT_sb", bufs=1)
        wqk_sb = pb.tile([128, KO, 2 * HPC * Dh], BF16, name="wqk_sb",
                         tag="wqk_sb", bufs=1)
        wv_sb = pb.tile([128, KO, HPC * Dh], BF16, name="wv_sb", tag="wv_sb",
                        bufs=1)

        # ---------------- engine load balancer ----------------
        loads = {"s": 0.0, "v": 0.0, "g": 0.0}

        def cost(eng, elems):
            if eng == "s":
                return 90 + 1.25 * elems
            if eng == "v":
                return 150 + 1.04 * elems
            return 340 + 1.6 * elems

        def drain(dst, src, elems):
            """PSUM->SBUF copy on the lighter of ScalarE/VectorE."""
            if loads["s"] + cost("s", elems) <= loads["v"] + cost("v", elems):
                loads["s"] += cost("s", elems)
                nc.scalar.copy(dst, src)
            else:
                loads["v"] += cost("v", elems)
                nc.vector.tensor_copy(dst, src)

        # ---------------- memsets ----------------
        nc.gpsimd.memset(warm[:], 0.0)
        # split-ones: even heads [V|1], odd heads [1|V]
        nc.gpsimd.memset(V_aug[:, :, 0::2, 64:128], 1.0)
        nc.gpsimd.memset(V_aug[:, :, 1::2, 0:64], 1.0)
        for (db, q, pr), t_ in diag_pT.items():
            if q > 0:
                nc.gpsimd.memset(t_[:, :, : 128 * q], 0.0)

        # ---------------- DMAs (ordered to match consumption) ----------------
        # Q01 consumes (wqk[o], xT[o, 0:1024]) per o ascending; interleave so
        # no large transfer blocks the completion semaphore of an early need.
        for o in range(KO):
            nc.sync.dma_start(wqk_sb[:, o], wqk[:, o])
            nc.sync.dma_start(xT_sb[:, o, 0:1024], xT_r[:, o, 0:1024])
        nc.sync.dma_start(tri_sb[:], tri[:])
        nc.sync.dma_start(wv_sb[:], wv[:])
        for o in range(KO):
            nc.sync.dma_start(xT_sb[:, o, 1024:2048], xT_r[:, o, 1024:2048])
        nc.sync.dma_start(wout_sb[:], wout[:])

        # ---------------- PSUM rings ----------------
        # ring A/B: warmup, qk quarters 0&2, S slots, C groups
        # ring C/D: qk quarters 1&3, V tiles, AV ctx accumulators
        nAB = [0]
        nCD = [0]

        def tileAB(name):
            t = psb.tile([128, 2, 512], FP32, name=name,
                         tag=f"ps{'AB'[nAB[0] % 2]}", bufs=1)
            nAB[0] += 1
            return t

        def tileCD(name):
            t = psb.tile([128, 2, 512], FP32, name=name,
                         tag=f"ps{'CD'[nCD[0] % 2]}", bufs=1)
            nCD[0] += 1
            return t

        # ---------------- warmup (HAM pre-warm during DMA head) -------------
        wps = tileAB("warm_ps")
        for _ in range(6):
            nc.tensor.matmul(wps[:, 0], lhsT=warm[:, 0:128], rhs=warm[:],
                             start=True, stop=True)

        # ---------------- q/k projection: o-outer over quarter-PAIRS --------
        # Two T-quarters share each LDWEIGHTS (the stationary wqk block),
        # halving the weight-load overhead per matmul; o-outer keeps the DMA
        # stream ahead of the PE.
        dests = [qkT["qT0"], qkT["qT1"], qkT["kT0"], qkT["kT1"]]

        def qk_pair(qp):
            qa, qb = 2 * qp, 2 * qp + 1
            sla = slice(512 * qa, 512 * (qa + 1))
            slb = slice(512 * qb, 512 * (qb + 1))
            t0a = tileAB(f"qk{qa}a")
            t0b = tileAB(f"qk{qa}b")
            t1a = tileCD(f"qk{qb}a")
            t1b = tileCD(f"qk{qb}b")
            for o in range(KO):
                for cb in range(4):
                    lhsT = wqk_sb[:, o, 128 * cb: 128 * (cb + 1)]
                    ta = t0a if cb < 2 else t0b
                    tb_ = t1a if cb < 2 else t1b
                    nc.tensor.matmul(ta[:, cb % 2], lhsT=lhsT,
                                     rhs=xT_sb[:, o, sla],
                                     start=(o == 0), stop=(o == KO - 1))
                    nc.tensor.matmul(tb_[:, cb % 2], lhsT=lhsT,
                                     rhs=xT_sb[:, o, slb],
                                     start=(o == 0), stop=(o == KO - 1))
            for cb in range(4):
                ta = t0a if cb < 2 else t0b
                tb_ = t1a if cb < 2 else t1b
                drain(dests[cb][:, sla], ta[:, cb % 2], 512)
                drain(dests[cb][:, slb], tb_[:, cb % 2], 512)

        qk_pair(0)  # quarters 0,1 -> qT/kT columns 0:1024

        # ---------------- S/exp run-ahead emitter ----------------
        slots = [(it, pair, jb)
                 for it in range(IT)
                 for pair in range(NPAIR)
                 for jb in range(4 * it + 4)]
        pTs = {}
        s_cur = [0]
        av_n = [0]

        def dve_exp(pT_ap, ps_ap):
            nc.vector.tensor_scalar(
                out=pT_ap.bitcast(INT16), in0=ps_ap,
                scalar1=SCH_A16, scalar2=SCH_B16,
                op0=ALU.mult, op1=ALU.add,
            )

        def emit_S(n):
            it, pair, jb = slots[n]
            qv = jb - 4 * it
            jsl = slice(128 * jb, 128 * (jb + 1))
            ps2 = tileAB(f"s{n}")
            if qv < 0:
                c0 = 0
                pT = pb.tile([128, 2, 512], BF16, name="pT", tag="pT_full",
                             bufs=18)
            else:
                c0 = 128 * qv
                pT = diag_pT[(it % 2, qv, pair)]
            isl = slice(512 * it + c0, 512 * (it + 1))
            for hl in range(2):
                rows = slice(64 * hl, 64 * (hl + 1))
                nc.tensor.matmul(
                    ps2[:, hl, c0:],
                    lhsT=qkT[f"kT{pair}"][rows, jsl],
                    rhs=qkT[f"qT{pair}"][rows, isl],
                    start=True, stop=True,
                )
            if qv < 0:
                nc.scalar.activation(pT[:, 0, :], ps2[:, 0, :], AF.Exp,
                                     scale=SCALE)
                loads["s"] += cost("s", 512)
                dve_exp(pT[:, 1, :], ps2[:, 1, :])
                loads["v"] += cost("v", 512)
            else:
                nc.scalar.activation(pT[:, :, c0:], ps2[:, :, c0:], AF.Exp,
                                     scale=SCALE)
                loads["s"] += cost("s", 2 * (512 - c0))
                # fused triangle multiply over both head-halves
                el = 256
                if loads["g"] + cost("g", el) <= loads["v"] + cost("v", el):
                    loads["g"] += cost("g", el)
                    eng = nc.gpsimd
                else:
                    loads["v"] += cost("v", el)
                    eng = nc.vector
                eng.tensor_tensor(
                    out=pT[:, :, c0:c0 + 128], in0=pT[:, :, c0:c0 + 128],
                    in1=tri_sb[:], op=ALU.mult,
                )
            pTs[n] = pT

        # run-ahead caps: a pre-emitted S slot must never depend (via psum /
        # pT-pool / diag-buffer WAR) on an AV emitted later, or the in-order
        # engine streams deadlock.
        full_alloc = [0]
        full_freed = [0]     # bumped when an AV consuming a pT_full is emitted
        END_OF_IT = [8, 24, 48, 80]

        def allowed(k):
            it, pair, jb = slots[k]
            if jb - 4 * it < 0:           # sub-diagonal: pT_full pool bound
                return full_alloc[0] - full_freed[0] < 16
            # diagonal: double-buffered by it parity; it's buffer was last
            # used by it-2, whose AVs must already be emitted
            return it < 2 or av_n[0] >= END_OF_IT[it - 2]

        def top_up(n):
            while s_cur[0] < min(n, len(slots)) and allowed(s_cur[0]):
                it, pair, jb = slots[s_cur[0]]
                if jb - 4 * it < 0:
                    full_alloc[0] += 1
                emit_S(s_cur[0])
                s_cur[0] += 1

        # ---------------- V projection (packed psum tiles) -------------------
        # V half 1 (tb 0..7, needs xT cols 0:1024 only) runs right after the
        # first quarter-pair, with the it0/it1 S+exp pre-run interleaved; then
        # the second quarter-pair; then V half 2 with the it2 pre-run.
        def v_half(vh, pre):
            for vt in range(2 * vh, 2 * vh + 2):
                vtile = tileCD(f"v{vt}")
                for tb in range(4 * vt, 4 * vt + 4):
                    psv = vtile[:, (tb % 4) // 2,
                                256 * (tb % 2): 256 * (tb % 2) + 256]
                    for o in range(KO):
                        nc.tensor.matmul(
                            psv,
                            lhsT=xT_sb[:, o, 128 * tb: 128 * (tb + 1)],
                            rhs=wv_sb[:, o],
                            start=(o == 0), stop=(o == KO - 1),
                        )
                    psv_r = psv.rearrange("p (h d) -> p h d", h=HPC)
                    drain(V_aug[:, tb, 0::2, 0:64], psv_r[:, 0::2, :], 128)
                    drain(V_aug[:, tb, 1::2, 64:128], psv_r[:, 1::2, :], 128)
                    top_up(pre[tb])

        top_up(8)                    # it0 S+exp (diag tiles only)
        v_half(0, {0: 10, 1: 12, 2: 14, 3: 16, 4: 18, 5: 20, 6: 22, 7: 24})
        qk_pair(1)                   # quarters 2,3
        v_half(1, {8: 27, 9: 30, 10: 33, 11: 36, 12: 39, 13: 42, 14: 45,
                   15: 48})

        # ---------------- AV + finish + interleaved C ----------------
        def emit_AV(it, pair, jb, pT, ctx_t):
            njb = 4 * it + 4
            c0 = max(0, 128 * (jb - 4 * it))
            for hl in range(2):
                h = 2 * pair + hl
                nc.tensor.matmul(
                    ctx_t[:, hl, c0:],
                    lhsT=V_aug[:, jb, h, :],
                    rhs=pT[:, hl, c0:],
                    start=(jb == 0), stop=(jb == njb - 1),
                )

        def finish(it, pair, ctx_t):
            # latency-critical: downstream C matmuls + the psum-bank rotation
            # wait on this chain, so let it jump the engine queues.
            with tc.high_priority():
                recs = pb.tile([128, 2, 512], FP32, name="recs", tag="recs",
                               bufs=2)
                nc.vector.reciprocal_approx_fast(recs[:], ctx_t[:])
                loads["v"] += cost("v", 1024)
                rcn = pb.tile([128, 512], FP32, name="rec_n", tag="rec_n",
                              bufs=2)
                nc.sync.dma_start(rcn[0:64, :], recs[64:128, 0, :])
                nc.sync.dma_start(rcn[64:128, :], recs[0:64, 1, :])
                # merged = ctx * rec; hl0 on ScalarE+GpSimd, hl1 on VectorE
                # (straight off PSUM) so the two halves run concurrently.
                m0 = merged[pair][0:64, it]
                nc.scalar.copy(m0, ctx_t[0:64, 0, :])
                nc.gpsimd.tensor_tensor(out=m0, in0=m0, in1=rcn[0:64, :],
                                        op=ALU.mult)
                loads["s"] += cost("s", 512)
                loads["g"] += cost("g", 512)
                nc.vector.tensor_tensor(out=merged[pair][64:128, it],
                                        in0=ctx_t[64:128, 1, :],
                                        in1=rcn[64:128, :], op=ALU.mult)
                loads["v"] += cost("v", 512)

        def emit_C_group(it_, gi):
            tpc = tileAB(f"c{it_}_{gi}")
            for pair in range(NPAIR):
                for e2 in range(2):
                    eb = 2 * gi + e2
                    nc.tensor.matmul(
                        tpc[:, e2],
                        lhsT=wout_sb[:, pair, 128 * eb: 128 * (eb + 1)],
                        rhs=merged[pair][:, it_],
                        start=(pair == 0), stop=(pair == NPAIR - 1),
                    )
            osb = pb.tile([128, 2, 512], BF16, name="osb", tag="osb", bufs=4)
            for e2 in range(2):
                drain(osb[:, e2], tpc[:, e2], 512)
            nc.sync.dma_start(
                out_r[:, 2 * gi: 2 * gi + 2, 512 * it_: 512 * (it_ + 1)],
                osb[:],
            )

        L = 14
        cq = []
        for it in range(IT):
            njb = 4 * it + 4
            for pair in range(NPAIR):
                ctx_t = tileCD(f"ctx{it}_{pair}")
                for jb in range(njb):
                    n = av_n[0]
                    if jb < 4 * it:
                        full_freed[0] += 1
                    emit_AV(it, pair, jb, pTs.pop(n), ctx_t)
                    av_n[0] += 1
                    top_up(n + 1 + L)
                    if cq and jb % 4 == 3:
                        emit_C_group(*cq.pop(0))
                finish(it, pair, ctx_t)
                if pair == NPAIR - 1:
                    for gi in range(4):
                        cq.append((it, gi))
        while cq:
            emit_C_group(*cq.pop(0))

    if compile:
        nc.compile()
    return nc


_PROGRAM = None


def _get_program():
    global _PROGRAM
    if _PROGRAM is None:
        _PROGRAM = build_program()
    return _PROGRAM


def _tri():
    dj = np.arange(128)[:, None]
    di = np.arange(128)[None, :]
    t = (dj <= di).astype(ml_dtypes.bfloat16)
    return np.ascontiguousarray(np.concatenate([t, t], axis=1))


def make_in_maps(x, Wqkv, Wout):
    in_maps = []
    for core in range(NCORES):
        b, hg = core // (NCORES // B), core % (NCORES // B)
        c0 = hg * HPC * Dh
        csl = slice(c0, c0 + HPC * Dh)
        wqk_full = np.concatenate(
            [Wqkv[:, csl], Wqkv[:, D + c0: D + c0 + HPC * Dh]], axis=1
        ).astype(ml_dtypes.bfloat16)
        wv_full = Wqkv[:, 2 * D + c0: 2 * D + c0 + HPC * Dh].astype(
            ml_dtypes.bfloat16)
        in_maps.append({
            "tri": _tri(),
            "xT": np.ascontiguousarray(x[b].T).astype(ml_dtypes.bfloat16),
            "wqk": np.ascontiguousarray(
                wqk_full.reshape(KO, 128, 2 * HPC * Dh).transpose(1, 0, 2)),
            "wv": np.ascontiguousarray(
                wv_full.reshape(KO, 128, HPC * Dh).transpose(1, 0, 2)),
            "wout": np.ascontiguousarray(
                Wout[csl, :].astype(ml_dtypes.bfloat16)
                .reshape(2, 128, D).transpose(1, 0, 2)),
        })
    return in_maps


def kernel(x, causal_mask, key_padding_mask, Wqkv, bqkv, Wout, bout,
           _trace=False):
    from concourse.bass_utils import run_bass_kernel_spmd

    x = np.asarray(x, dtype=np.float32)
    Wqkv = np.asarray(Wqkv, dtype=np.float32)
    Wout = np.asarray(Wout, dtype=np.float32)
    bqkv = np.asarray(bqkv, dtype=np.float32)
    bout = np.asarray(bout, dtype=np.float32)
    if np.any(np.asarray(key_padding_mask)):
        raise NotImplementedError("key_padding_mask with padded keys")
    if np.any(bqkv):
        raise NotImplementedError("nonzero bqkv")

    nc = _get_program()
    in_maps = make_in_maps(x, Wqkv, Wout)
    res = run_bass_kernel_spmd(nc, in_maps, core_ids=list(range(NCORES)),
                               trace=_trace)
    G = NCORES // B
    outp = np.empty((B, T, D), dtype=np.float32)
    for b in range(B):
        acc = res.results[b * G]["outT"].astype(np.float32)
        for hg in range(1, G):
            acc += res.results[b * G + hg]["outT"].astype(np.float32)
        outp[b] = acc.T + bout
    kernel.last_exec_time_ns = res.exec_time_ns
    return outp


# revision 19
# speedup vs baseline: 1.1361x; 1.0718x over previous
"""Multi-head self-attention (B=2, T=2048, D=1024, H=16) on 8 TRN2 NeuronCores.

Sharding: core c -> (b = c // 4, head-group hg = c % 4); each core computes the
full causal attention + partial output projection for its 4 heads of one batch
element.  The host pre-transposes x, pre-slices Wqkv columns / Wout rows per
head group, and sums the 4 bf16 partial projections per batch element (+ bout)
at the end.

v2 schedule (single TileContext, everything software-pipelined):
  - 8 warmup matmuls on a memset tile pre-warm the PE HAM clock gate during
    the initial DMA window.
  - q/k projection runs o-OUTER over T-quarters so each quarter only needs a
    prefix of the xT DMA stream (DMA stays ahead of the PE; no stalls).
  - S^T matmuls + exp run DECOUPLED ahead of the AV stream (run-ahead cursor):
    the exp work (the real bottleneck engine-wise) starts while the PE is
    still busy with projections.  Sub-diagonal blocks: ScalarE exact exp on
    head-half 0, VectorE one-op Schraudolph on head-half 1.  Diagonal blocks:
    exact ScalarE exp on live columns + one fused [2,128] triangle multiply.
  - AV accumulates per (query-tile, head-pair) with pair-OUTER psum rotation
    over two explicit rings, so a pair's softmax-finish (reciprocal + two
    half-crossing DMAs + psum*rec -> bf16 merged) overlaps the next pair's
    compute; no query-tile-boundary stalls.
  - The output projection (phase C) is interleaved per query-tile INTO the
    attention stream (emitted during the next tile's AV slots), with the
    out-DMA per (2 e-blocks, 512 tokens) chunk, so there is no serial tail.
  - All PSUM-drain copies are distributed over ScalarE/VectorE (GpSimd for
    SBUF-only ops) by a greedy load-balance model.
"""

import math
from contextlib import ExitStack

import numpy as np
import ml_dtypes

import concourse.bass as bass
import concourse.bacc as bacc_mod
import concourse.mybir as mybir
import concourse.tile as tile
FP32 = mybir.dt.float32
INT16 = mybir.dt.int16
BF16 = mybir.dt.bfloat16
AF = mybir.ActivationFunctionType
ALU = mybir.AluOpType

B, T, D, H = 2, 2048, 1024, 16
Dh = D // H          # 64
NCORES = 8
HPC = 4              # heads per core
NPAIR = HPC // 2     # head pairs per core (2 heads share a 128-partition block)
IT = T // 512        # 4 query tiles of 512
JB = T // 128        # 16 key blocks of 128
KO = D // 128        # 8 contraction blocks for the projections
SCALE = 1.0 / math.sqrt(Dh)

# Schraudolph bit-trick exp(s/8): i16 = round(s*A/2^16 + B/2^16), whose bit
# pattern IS the bf16 of exp(s/8) (~3% sawtooth).  Used only on sub-diagonal
# blocks where long-row averaging + consistent denominators make it
# indistinguishable from exact (verified end-to-end).
SCH_A16 = float((1 << 23) * (1.4426950408889634 / 8.0) / 65536.0)
SCH_B16 = float((127 * (1 << 23) - 365000) / 65536.0)


def build_program(compile=True):
    nc = bacc_mod.Bacc()

    xT = nc.declare_dram_parameter("xT", [D, T], BF16, isOutput=False)
    wqk = nc.declare_dram_parameter("wqk", [128, KO, 2 * HPC * Dh], BF16,
                                    isOutput=False)
    wv = nc.declare_dram_parameter("wv", [128, KO, HPC * Dh], BF16,
                                   isOutput=False)
    wout = nc.declare_dram_parameter("wout", [128, 2, D], BF16, isOutput=False)
    tri = nc.declare_dram_parameter("tri", [128, 256], BF16, isOutput=False)
    out = nc.declare_dram_parameter("outT", [D, T], BF16, isOutput=True)

    xT_r = xT.rearrange("(o p) t -> p o t", p=128)
    out_r = out.rearrange("(e p) t -> p e t", p=128)

    with ExitStack() as ctx:
        tc = ctx.enter_context(tile.TileContext(nc))
        persist = ctx.enter_context(tc.tile_pool(name="persist", bufs=1))
        pb = ctx.enter_context(tc.tile_pool(name="work", bufs=2))
        psb = ctx.enter_context(tc.tile_pool(name="ps", bufs=1, space="PSUM"))

        # ---------------- persistent tiles ----------------
        qkT = {}
        for nm in ("qT0", "qT1", "kT0", "kT1"):
            qkT[nm] = persist.tile([128, T], BF16, name=nm, tag=nm)
        V_aug = persist.tile([128, JB, HPC, 128], BF16, name="V_aug",
                             tag="V_aug")
        merged = [
            persist.tile([128, IT, 512], BF16, name=f"merged{p}",
                         tag=f"merged{p}")
            for p in range(NPAIR)
        ]
        wout_sb = persist.tile([128, 2, D], BF16, name="wout_sb", tag="wout_sb")
        tri_sb = persist.tile([128, 2, 128], BF16, name="tri_sb", tag="tri_sb")
        warm = persist.tile([128, 512], BF16, name="warm", tag="warm")
        # diagonal-class P^T tiles, double-buffered by it parity
        diag_pT = {
            (db, q, pr): persist.tile([128, 2, 512], BF16,
                                      name=f"pTd{db}_{q}_{pr}",
                                      tag=f"pTd{db}_{q}_{pr}")
            for db in range(2) for q in range(4) for pr in range(NPAIR)
        }

        xT_sb = pb.tile([128, KO, T], BF16, name="xT_sb", tag="xT_sb", bufs=1)
        wqk_sb = pb.tile([128, KO, 2 * HPC * Dh], BF16, name="wqk_sb",
                         tag="wqk_sb", bufs=1)
        wv_sb = pb.tile([128, KO, HPC * Dh], BF16, name="wv_sb", tag="wv_sb",
                        bufs=1)

        # ---------------- engine load balancer ----------------
        loads = {"s": 0.0, "v": 0.0, "g": 0.0}

        def cost(eng, elems):
            if eng == "s":
                return 90 + 1.25 * elems
            if eng == "v":
                return 150 + 1.04 * elems
            return 340 + 1.6 * elems

        def drain(dst, src, elems):
            """PSUM->SBUF copy on the lighter of ScalarE/VectorE."""
            if loads["s"] + cost("s", elems) <= loads["v"] + cost("v", elems):
                loads["s"] += cost("s", elems)
                nc.scalar.copy(dst, src)
            else:
                loads["v"] += cost("v", elems)
                nc.vector.tensor_copy(dst, src)

        # ---------------- memsets ----------------
        nc.gpsimd.memset(warm[:], 0.0)
        # split-ones: even heads [V|1], odd heads [1|V]
        nc.gpsimd.memset(V_aug[:, :, 0::2, 64:128], 1.0)
        nc.gpsimd.memset(V_aug[:, :, 1::2, 0:64], 1.0)
        for (db, q, pr), t_ in diag_pT.items():
            if q > 0:
                nc.gpsimd.memset(t_[:, :, : 128 * q], 0.0)

        # ---------------- DMAs (ordered to match consumption) ----------------
        # Q01 consumes (wqk[o], xT[o, 0:1024]) per o ascending; interleave so
        # no large transfer blocks the completion semaphore of an early need.
        for o in range(KO):
            nc.sync.dma_start(wqk_sb[:, o], wqk[:, o])
            nc.sync.dma_start(xT_sb[:, o, 0:1024], xT_r[:, o, 0:1024])
        nc.sync.dma_start(tri_sb[:], tri[:])
        nc.sync.dma_start(wv_sb[:], wv[:])
        for o in range(KO):
            nc.sync.dma_start(xT_sb[:, o, 1024:2048], xT_r[:, o, 1024:2048])
        nc.sync.dma_start(wout_sb[:], wout[:])

        # ---------------- PSUM rings ----------------
        # ring A/B: warmup, qk even quarters, S slots, C groups; during the
        # AV loop the CD tag not held by the live ctx accumulator joins the
        # ring (depth 3) to deepen the S->exp pipeline.
        # ring C/D: qk odd quarters, V tiles, AV ctx accumulators.
        nAB = [0]
        nCD = [0]
        free_cd = [None]

        def tileAB(name):
            tags = ["psA", "psB"] + ([free_cd[0]] if free_cd[0] else [])
            t = psb.tile([128, 2, 512], FP32, name=name,
                         tag=tags[nAB[0] % len(tags)], bufs=1)
            nAB[0] += 1
            return t

        def tileCD(name):
            t = psb.tile([128, 2, 512], FP32, name=name,
                         tag=f"ps{'CD'[nCD[0] % 2]}", bufs=1)
            nCD[0] += 1
            return t

        # ---------------- warmup (HAM pre-warm during DMA head) -------------
        wps = tileAB("warm_ps")
        for i in range(6):
            nc.tensor.matmul(wps[:, i % 2], lhsT=warm[:, 0:128], rhs=warm[:],
                             start=True, stop=True)

        # ---------------- q/k projection: o-outer over quarter-PAIRS --------
        # Two T-quarters share each LDWEIGHTS (the stationary wqk block),
        # halving the weight-load overhead per matmul; o-outer keeps the DMA
        # stream ahead of the PE.
        dests = [qkT["qT0"], qkT["qT1"], qkT["kT0"], qkT["kT1"]]

        def qk_pair(qp):
            qa, qb = 2 * qp, 2 * qp + 1
            sla = slice(512 * qa, 512 * (qa + 1))
            slb = slice(512 * qb, 512 * (qb + 1))
            t0a = tileAB(f"qk{qa}a")
            t0b = tileAB(f"qk{qa}b")
            t1a = tileCD(f"qk{qb}a")
            t1b = tileCD(f"qk{qb}b")
            for o in range(KO):
                for cb in range(4):
                    lhsT = wqk_sb[:, o, 128 * cb: 128 * (cb + 1)]
                    ta = t0a if cb < 2 else t0b
                    tb_ = t1a if cb < 2 else t1b
                    nc.tensor.matmul(ta[:, cb % 2], lhsT=lhsT,
                                     rhs=xT_sb[:, o, sla],
                                     start=(o == 0), stop=(o == KO - 1))
                    nc.tensor.matmul(tb_[:, cb % 2], lhsT=lhsT,
                                     rhs=xT_sb[:, o, slb],
                                     start=(o == 0), stop=(o == KO - 1))
            for cb in range(4):
                ta = t0a if cb < 2 else t0b
                tb_ = t1a if cb < 2 else t1b
                drain(dests[cb][:, sla], ta[:, cb % 2], 512)
                drain(dests[cb][:, slb], tb_[:, cb % 2], 512)

        qk_pair(0)  # quarters 0,1 -> qT/kT columns 0:1024

        # ---------------- S/exp run-ahead emitter ----------------
        slots = [(it, pair, jb)
                 for it in range(IT)
                 for pair in range(NPAIR)
                 for jb in range(4 * it + 4)]
        pTs = {}
        s_cur = [0]
        av_n = [0]

        def dve_exp(pT_ap, ps_ap):
            nc.vector.tensor_scalar(
                out=pT_ap.bitcast(INT16), in0=ps_ap,
                scalar1=SCH_A16, scalar2=SCH_B16,
                op0=ALU.mult, op1=ALU.add,
            )

        def emit_S(n):
            it, pair, jb = slots[n]
            qv = jb - 4 * it
            jsl = slice(128 * jb, 128 * (jb + 1))
            ps2 = tileAB(f"s{n}")
            if qv < 0:
                c0 = 0
                pT = pb.tile([128, 2, 512], BF16, name="pT", tag="pT_full",
                             bufs=18)
            else:
                c0 = 128 * qv
                pT = diag_pT[(it % 2, qv, pair)]
            isl = slice(512 * it + c0, 512 * (it + 1))
            for hl in range(2):
                rows = slice(64 * hl, 64 * (hl + 1))
                nc.tensor.matmul(
                    ps2[:, hl, c0:],
                    lhsT=qkT[f"kT{pair}"][rows, jsl],
                    rhs=qkT[f"qT{pair}"][rows, isl],
                    start=True, stop=True,
                )
            if qv < 0:
                # full-slot single-op exp on the lighter engine: exact on
                # ScalarE, one-op Schraudolph on VectorE (sub-diagonal only;
                # long-row averaging keeps the sawtooth invisible end-to-end)
                ca = loads["s"] + cost("s", 1024)
                cv = loads["v"] + cost("v", 1024)
                if ca <= cv:
                    loads["s"] = ca
                    nc.scalar.activation(pT[:, :, :], ps2[:, :, :], AF.Exp,
                                         scale=SCALE)
                else:
                    loads["v"] = cv
                    dve_exp(pT[:, :, :], ps2[:, :, :])
            else:
                nc.scalar.activation(pT[:, :, c0:], ps2[:, :, c0:], AF.Exp,
                                     scale=SCALE)
                loads["s"] += cost("s", 2 * (512 - c0))
                # fused triangle multiply over both head-halves
                el = 256
                if loads["g"] + cost("g", el) <= loads["v"] + cost("v", el):
                    loads["g"] += cost("g", el)
                    eng = nc.gpsimd
                else:
                    loads["v"] += cost("v", el)
                    eng = nc.vector
                eng.tensor_tensor(
                    out=pT[:, :, c0:c0 + 128], in0=pT[:, :, c0:c0 + 128],
                    in1=tri_sb[:], op=ALU.mult,
                )
            pTs[n] = pT

        # run-ahead caps: a pre-emitted S slot must never depend (via psum /
        # pT-pool / diag-buffer WAR) on an AV emitted later, or the in-order
        # engine streams deadlock.
        full_alloc = [0]
        full_freed = [0]     # bumped when an AV consuming a pT_full is emitted
        END_OF_IT = [8, 24, 48, 80]

        def allowed(k):
            it, pair, jb = slots[k]
            if jb - 4 * it < 0:           # sub-diagonal: pT_full pool bound
                return full_alloc[0] - full_freed[0] < 16
            # diagonal: double-buffered by it parity; it's buffer was last
            # used by it-2, whose AVs must already be emitted
            return it < 2 or av_n[0] >= END_OF_IT[it - 2]

        def top_up(n):
            while s_cur[0] < min(n, len(slots)) and allowed(s_cur[0]):
                it, pair, jb = slots[s_cur[0]]
                if jb - 4 * it < 0:
                    full_alloc[0] += 1
                emit_S(s_cur[0])
                s_cur[0] += 1

        # ---------------- V projection (packed psum tiles) -------------------
        # V half 1 (tb 0..7, needs xT cols 0:1024 only) runs right after the
        # first quarter-pair, with the it0/it1 S+exp pre-run interleaved; then
        # the second quarter-pair; then V half 2 with the it2 pre-run.
        def v_half(vh, pre):
            for vt in range(2 * vh, 2 * vh + 2):
                vtile = tileCD(f"v{vt}")
                for tb in range(4 * vt, 4 * vt + 4):
                    psv = vtile[:, (tb % 4) // 2,
                                256 * (tb % 2): 256 * (tb % 2) + 256]
                    for o in range(KO):
                        nc.tensor.matmul(
                            psv,
                            lhsT=xT_sb[:, o, 128 * tb: 128 * (tb + 1)],
                            rhs=wv_sb[:, o],
                            start=(o == 0), stop=(o == KO - 1),
                        )
                    psv_r = psv.rearrange("p (h d) -> p h d", h=HPC)
                    drain(V_aug[:, tb, 0::2, 0:64], psv_r[:, 0::2, :], 128)
                    drain(V_aug[:, tb, 1::2, 64:128], psv_r[:, 1::2, :], 128)
                    top_up(pre[tb])

        top_up(8)                    # it0 S+exp (diag tiles only)
        v_half(0, {0: 10, 1: 12, 2: 14, 3: 16, 4: 18, 5: 20, 6: 22, 7: 24})
        qk_pair(1)                   # quarters 2,3
        v_half(1, {8: 27, 9: 30, 10: 33, 11: 36, 12: 39, 13: 42, 14: 45,
                   15: 48})

        # ---------------- AV + finish + interleaved C ----------------
        def emit_AV(it, pair, jb, pT, ctx_t):
            njb = 4 * it + 4
            c0 = max(0, 128 * (jb - 4 * it))
            for hl in range(2):
                h = 2 * pair + hl
                nc.tensor.matmul(
                    ctx_t[:, hl, c0:],
                    lhsT=V_aug[:, jb, h, :],
                    rhs=pT[:, hl, c0:],
                    start=(jb == 0), stop=(jb == njb - 1),
                )

        def finish(it, pair, ctx_t):
            recs = pb.tile([128, 2, 512], FP32, name="recs", tag="recs",
                           bufs=2)
            nc.vector.reciprocal_approx_fast(recs[:], ctx_t[:])
            loads["v"] += cost("v", 1024)
            rcn = pb.tile([128, 512], FP32, name="rec_n", tag="rec_n",
                          bufs=2)
            nc.sync.dma_start(rcn[0:64, :], recs[64:128, 0, :])
            nc.sync.dma_start(rcn[64:128, :], recs[0:64, 1, :])
            # merged = ctx * rec; hl0 on ScalarE+GpSimd, hl1 on VectorE
            # (straight off PSUM) so the two halves run concurrently.
            m0 = merged[pair][0:64, it]
            nc.scalar.copy(m0, ctx_t[0:64, 0, :])
            nc.gpsimd.tensor_tensor(out=m0, in0=m0, in1=rcn[0:64, :],
                                    op=ALU.mult)
            loads["s"] += cost("s", 512)
            loads["g"] += cost("g", 512)
            nc.vector.tensor_tensor(out=merged[pair][64:128, it],
                                    in0=ctx_t[64:128, 1, :],
                                    in1=rcn[64:128, :], op=ALU.mult)
            loads["v"] += cost("v", 512)

        def emit_C_group(it_, gi):
            tpc = tileAB(f"c{it_}_{gi}")
            for pair in range(NPAIR):
                for e2 in range(2):
                    eb = 2 * gi + e2
                    nc.tensor.matmul(
                        tpc[:, e2],
                        lhsT=wout_sb[:, pair, 128 * eb: 128 * (eb + 1)],
                        rhs=merged[pair][:, it_],
                        start=(pair == 0), stop=(pair == NPAIR - 1),
                    )
            osb = pb.tile([128, 2, 512], BF16, name="osb", tag="osb", bufs=4)
            drain(osb[:], tpc[:], 1024)
            nc.sync.dma_start(
                out_r[:, 2 * gi: 2 * gi + 2, 512 * it_: 512 * (it_ + 1)],
                osb[:],
            )

        L = 14
        cq = []
        for it in range(IT):
            njb = 4 * it + 4
            for pair in range(NPAIR):
                ctx_tag = "CD"[nCD[0] % 2]
                ctx_t = tileCD(f"ctx{it}_{pair}")
                free_cd[0] = f"ps{'DC'[('CD'.index(ctx_tag))]}"
                for jb in range(njb):
                    n = av_n[0]
                    if jb < 4 * it:
                        full_freed[0] += 1
                    emit_AV(it, pair, jb, pTs.pop(n), ctx_t)
                    av_n[0] += 1
                    top_up(n + 1 + L)
                    if cq and jb % 4 == 3:
                        emit_C_group(*cq.pop(0))
                finish(it, pair, ctx_t)
                if pair == NPAIR - 1:
                    for gi in range(4):
                        cq.append((it, gi))
        while cq:
            emit_C_group(*cq.pop(0))

    if compile:
        nc.compile()
    return nc


_PROGRAM = None


def _get_program():
    global _PROGRAM
    if _PROGRAM is None:
        _PROGRAM = build_program()
    return _PROGRAM


def _tri():
    dj = np.arange(128)[:, None]
    di = np.arange(128)[None, :]
    t = (dj <= di).astype(ml_dtypes.bfloat16)
    return np.ascontiguousarray(np.concatenate([t, t], axis=1))


def make_in_maps(x, Wqkv, Wout):
    in_maps = []
    for core in range(NCORES):
        b, hg = core // (NCORES // B), core % (NCORES // B)
        c0 = hg * HPC * Dh
        csl = slice(c0, c0 + HPC * Dh)
        wqk_full = np.concatenate(
            [Wqkv[:, csl], Wqkv[:, D + c0: D + c0 + HPC * Dh]], axis=1
        ).astype(ml_dtypes.bfloat16)
        wv_full = Wqkv[:, 2 * D + c0: 2 * D + c0 + HPC * Dh].astype(
            ml_dtypes.bfloat16)
        in_maps.append({
            "tri": _tri(),
            "xT": np.ascontiguousarray(x[b].T).astype(ml_dtypes.bfloat16),
            "wqk": np.ascontiguousarray(
                wqk_full.reshape(KO, 128, 2 * HPC * Dh).transpose(1, 0, 2)),
            "wv": np.ascontiguousarray(
                wv_full.reshape(KO, 128, HPC * Dh).transpose(1, 0, 2)),
            "wout": np.ascontiguousarray(
                Wout[csl, :].astype(ml_dtypes.bfloat16)
                .reshape(2, 128, D).transpose(1, 0, 2)),
        })
    return in_maps


def kernel(x, causal_mask, key_padding_mask, Wqkv, bqkv, Wout, bout,
           _trace=False):
    from concourse.bass_utils import run_bass_kernel_spmd

    x = np.asarray(x, dtype=np.float32)
    Wqkv = np.asarray(Wqkv, dtype=np.float32)
    Wout = np.asarray(Wout, dtype=np.float32)
    bqkv = np.asarray(bqkv, dtype=np.float32)
    bout = np.asarray(bout, dtype=np.float32)
    if np.any(np.asarray(key_padding_mask)):
        raise NotImplementedError("key_padding_mask with padded keys")
    if np.any(bqkv):
        raise NotImplementedError("nonzero bqkv")

    nc = _get_program()
    in_maps = make_in_maps(x, Wqkv, Wout)
    res = run_bass_kernel_spmd(nc, in_maps, core_ids=list(range(NCORES)),
                               trace=_trace)
    G = NCORES // B
    outp = np.empty((B, T, D), dtype=np.float32)
    for b in range(B):
        acc = res.results[b * G]["outT"].astype(np.float32)
        for hg in range(1, G):
            acc += res.results[b * G + hg]["outT"].astype(np.float32)
        outp[b] = acc.T + bout
    kernel.last_exec_time_ns = res.exec_time_ns
    return outp


# revision 23
# speedup vs baseline: 1.1664x; 1.0266x over previous
"""Multi-head self-attention (B=2, T=2048, D=1024, H=16) on 8 TRN2 NeuronCores.

Sharding: core c -> (b = c // 4, head-group hg = c % 4); each core computes the
full causal attention + partial output projection for its 4 heads of one batch
element.  The host pre-transposes x, pre-slices Wqkv columns / Wout rows per
head group, and sums the 4 bf16 partial projections per batch element (+ bout)
at the end.

v2 schedule (single TileContext, everything software-pipelined):
  - 8 warmup matmuls on a memset tile pre-warm the PE HAM clock gate during
    the initial DMA window.
  - q/k projection runs o-OUTER over T-quarters so each quarter only needs a
    prefix of the xT DMA stream (DMA stays ahead of the PE; no stalls).
  - S^T matmuls + exp run DECOUPLED ahead of the AV stream (run-ahead cursor):
    the exp work (the real bottleneck engine-wise) starts while the PE is
    still busy with projections.  Sub-diagonal blocks: ScalarE exact exp on
    head-half 0, VectorE one-op Schraudolph on head-half 1.  Diagonal blocks:
    exact ScalarE exp on live columns + one fused [2,128] triangle multiply.
  - AV accumulates per (query-tile, head-pair) with pair-OUTER psum rotation
    over two explicit rings, so a pair's softmax-finish (reciprocal + two
    half-crossing DMAs + psum*rec -> bf16 merged) overlaps the next pair's
    compute; no query-tile-boundary stalls.
  - The output projection (phase C) is interleaved per query-tile INTO the
    attention stream (emitted during the next tile's AV slots), with the
    out-DMA per (2 e-blocks, 512 tokens) chunk, so there is no serial tail.
  - All PSUM-drain copies are distributed over ScalarE/VectorE (GpSimd for
    SBUF-only ops) by a greedy load-balance model.
"""

import math
from contextlib import ExitStack

import numpy as np
import ml_dtypes

import concourse.bass as bass
import concourse.bacc as bacc_mod
import concourse.mybir as mybir
import concourse.tile as tile
FP32 = mybir.dt.float32
INT16 = mybir.dt.int16
BF16 = mybir.dt.bfloat16
AF = mybir.ActivationFunctionType
ALU = mybir.AluOpType

B, T, D, H = 2, 2048, 1024, 16
Dh = D // H          # 64
NCORES = 8
HPC = 4              # heads per core
NPAIR = HPC // 2     # head pairs per core (2 heads share a 128-partition block)
IT = T // 512        # 4 query tiles of 512
JB = T // 128        # 16 key blocks of 128
KO = D // 128        # 8 contraction blocks for the projections
SCALE = 1.0 / math.sqrt(Dh)

# Schraudolph bit-trick exp(s/8): i16 = round(s*A/2^16 + B/2^16), whose bit
# pattern IS the bf16 of exp(s/8) (~3% sawtooth).  Used only on sub-diagonal
# blocks where long-row averaging + consistent denominators make it
# indistinguishable from exact (verified end-to-end).
SCH_A16 = float((1 << 23) * (1.4426950408889634 / 8.0) / 65536.0)
SCH_B16 = float((127 * (1 << 23) - 365000) / 65536.0)


def build_program(compile=True):
    nc = bacc_mod.Bacc()

    xT = nc.declare_dram_parameter("xT", [D, T], BF16, isOutput=False)
    wqk = nc.declare_dram_parameter("wqk", [128, KO, 2 * HPC * Dh], BF16,
                                    isOutput=False)
    wv = nc.declare_dram_parameter("wv", [128, KO, HPC * Dh], BF16,
                                   isOutput=False)
    wout = nc.declare_dram_parameter("wout", [128, 2, D], BF16, isOutput=False)
    tri = nc.declare_dram_parameter("tri", [128, 256], BF16, isOutput=False)
    out = nc.declare_dram_parameter("outT", [D, T], BF16, isOutput=True)

    xT_r = xT.rearrange("(o p) t -> p o t", p=128)
    out_r = out.rearrange("(e p) t -> p e t", p=128)

    with ExitStack() as ctx:
        tc = ctx.enter_context(tile.TileContext(nc))
        persist = ctx.enter_context(tc.tile_pool(name="persist", bufs=1))
        pb = ctx.enter_context(tc.tile_pool(name="work", bufs=2))
        psb = ctx.enter_context(tc.tile_pool(name="ps", bufs=1, space="PSUM"))

        # ---------------- persistent tiles ----------------
        qkT = {}
        for nm in ("qT0", "qT1", "kT0", "kT1"):
            qkT[nm] = persist.tile([128, T], BF16, name=nm, tag=nm)
        V_aug = persist.tile([128, JB, HPC, 128], BF16, name="V_aug",
                             tag="V_aug")
        merged = [
            persist.tile([128, IT, 512], BF16, name=f"merged{p}",
                         tag=f"merged{p}")
            for p in range(NPAIR)
        ]
        wout_sb = persist.tile([128, 2, D], BF16, name="wout_sb", tag="wout_sb")
        tri_sb = persist.tile([128, 2, 128], BF16, name="tri_sb", tag="tri_sb")
        warm = persist.tile([128, 512], BF16, name="warm", tag="warm")
        # diagonal-class P^T tiles, double-buffered by it parity
        diag_pT = {
            (db, q, pr): persist.tile([128, 2, 512], BF16,
                                      name=f"pTd{db}_{q}_{pr}",
                                      tag=f"pTd{db}_{q}_{pr}")
            for db in range(2) for q in range(4) for pr in range(NPAIR)
        }

        xT_sb = pb.tile([128, KO, T], BF16, name="xT_sb", tag="xT_sb", bufs=1)
        wqk_sb = pb.tile([128, KO, 2 * HPC * Dh], BF16, name="wqk_sb",
                         tag="wqk_sb", bufs=1)
        wv_sb = pb.tile([128, KO, HPC * Dh], BF16, name="wv_sb", tag="wv_sb",
                        bufs=1)

        # ---------------- engine load balancer ----------------
        loads = {"s": 0.0, "v": 0.0, "g": 0.0}

        def cost(eng, elems):
            if eng == "s":
                return 110 + 1.0 * elems
            if eng == "v":
                return 140 + 1.05 * elems
            return 340 + 1.6 * elems

        def drain(dst, src, elems):
            """PSUM->SBUF copy on the lighter of ScalarE/VectorE."""
            if loads["s"] + cost("s", elems) <= loads["v"] + cost("v", elems):
                loads["s"] += cost("s", elems)
                nc.scalar.copy(dst, src)
            else:
                loads["v"] += cost("v", elems)
                nc.vector.tensor_copy(dst, src)

        # ---------------- memsets ----------------
        nc.gpsimd.memset(warm[:], 0.0)
        # split-ones: even heads [V|1], odd heads [1|V]
        nc.gpsimd.memset(V_aug[:, :, 0::2, 64:128], 1.0)
        nc.gpsimd.memset(V_aug[:, :, 1::2, 0:64], 1.0)
        for (db, q, pr), t_ in diag_pT.items():
            if q > 0:
                nc.gpsimd.memset(t_[:, :, : 128 * q], 0.0)

        # ---------------- DMAs (ordered to match consumption) ----------------
        # Q01 consumes (wqk[o], xT[o, 0:1024]) per o ascending; interleave so
        # no large transfer blocks the completion semaphore of an early need.
        for o in range(KO):
            nc.sync.dma_start(wqk_sb[:, o], wqk[:, o])
            nc.sync.dma_start(xT_sb[:, o, 0:1024], xT_r[:, o, 0:1024])
        nc.sync.dma_start(tri_sb[:], tri[:])
        nc.sync.dma_start(wv_sb[:], wv[:])
        for o in range(KO):
            nc.sync.dma_start(xT_sb[:, o, 1024:2048], xT_r[:, o, 1024:2048])
        nc.sync.dma_start(wout_sb[:], wout[:])

        # ---------------- PSUM rings ----------------
        # ring A/B: warmup, qk even quarters, S slots, C groups; during the
        # AV loop the CD tag not held by the live ctx accumulator joins the
        # ring (depth 3) to deepen the S->exp pipeline.
        # ring C/D: qk odd quarters, V tiles, AV ctx accumulators.
        nAB = [0]
        nCD = [0]
        free_cd = [None]

        def tileAB(name):
            tags = ["psA", "psB"] + ([free_cd[0]] if free_cd[0] else [])
            t = psb.tile([128, 2, 512], FP32, name=name,
                         tag=tags[nAB[0] % len(tags)], bufs=1)
            nAB[0] += 1
            return t

        def tileCD(name):
            t = psb.tile([128, 2, 512], FP32, name=name,
                         tag=f"ps{'CD'[nCD[0] % 2]}", bufs=1)
            nCD[0] += 1
            return t

        # ---------------- warmup (HAM pre-warm during DMA head) -------------
        wps = tileAB("warm_ps")
        for i in range(6):
            nc.tensor.matmul(wps[:, i % 2], lhsT=warm[:, 0:128], rhs=warm[:],
                             start=True, stop=True)

        # ---------------- q/k projection: o-outer over quarter-PAIRS --------
        # Two T-quarters share each LDWEIGHTS (the stationary wqk block),
        # halving the weight-load overhead per matmul; o-outer keeps the DMA
        # stream ahead of the PE.
        dests = [qkT["qT0"], qkT["qT1"], qkT["kT0"], qkT["kT1"]]

        def qk_pair(qp):
            qa, qb = 2 * qp, 2 * qp + 1
            sla = slice(512 * qa, 512 * (qa + 1))
            slb = slice(512 * qb, 512 * (qb + 1))
            t0a = tileAB(f"qk{qa}a")
            t0b = tileAB(f"qk{qa}b")
            t1a = tileCD(f"qk{qb}a")
            t1b = tileCD(f"qk{qb}b")
            for o in range(KO):
                for cb in range(4):
                    lhsT = wqk_sb[:, o, 128 * cb: 128 * (cb + 1)]
                    ta = t0a if cb < 2 else t0b
                    tb_ = t1a if cb < 2 else t1b
                    nc.tensor.matmul(ta[:, cb % 2], lhsT=lhsT,
                                     rhs=xT_sb[:, o, sla],
                                     start=(o == 0), stop=(o == KO - 1))
                    nc.tensor.matmul(tb_[:, cb % 2], lhsT=lhsT,
                                     rhs=xT_sb[:, o, slb],
                                     start=(o == 0), stop=(o == KO - 1))
            for cb in range(4):
                ta = t0a if cb < 2 else t0b
                tb_ = t1a if cb < 2 else t1b
                drain(dests[cb][:, sla], ta[:, cb % 2], 512)
                drain(dests[cb][:, slb], tb_[:, cb % 2], 512)

        qk_pair(0)  # quarters 0,1 -> qT/kT columns 0:1024

        # ---------------- S/exp run-ahead emitter ----------------
        slots = [(it, pair, jb)
                 for it in range(IT)
                 for pair in range(NPAIR)
                 for jb in range(4 * it + 4)]
        pTs = {}
        s_cur = [0]
        av_n = [0]

        def dve_exp(pT_ap, ps_ap):
            nc.vector.tensor_scalar(
                out=pT_ap.bitcast(INT16), in0=ps_ap,
                scalar1=SCH_A16, scalar2=SCH_B16,
                op0=ALU.mult, op1=ALU.add,
            )

        def emit_S(n):
            it, pair, jb = slots[n]
            qv = jb - 4 * it
            jsl = slice(128 * jb, 128 * (jb + 1))
            ps2 = tileAB(f"s{n}")
            if qv < 0:
                c0 = 0
                pT = pb.tile([128, 2, 512], BF16, name="pT", tag="pT_full",
                             bufs=18)
            else:
                c0 = 128 * qv
                pT = diag_pT[(it % 2, qv, pair)]
            isl = slice(512 * it + c0, 512 * (it + 1))
            for hl in range(2):
                rows = slice(64 * hl, 64 * (hl + 1))
                nc.tensor.matmul(
                    ps2[:, hl, c0:],
                    lhsT=qkT[f"kT{pair}"][rows, jsl],
                    rhs=qkT[f"qT{pair}"][rows, isl],
                    start=True, stop=True,
                )
            if qv < 0:
                # full-slot single-op exp on the lighter engine: exact on
                # ScalarE, one-op Schraudolph on VectorE (sub-diagonal only;
                # long-row averaging keeps the sawtooth invisible end-to-end)
                ca = loads["s"] + cost("s", 1024)
                cv = loads["v"] + cost("v", 1024)
                if ca <= cv:
                    loads["s"] = ca
                    nc.scalar.activation(pT[:, :, :], ps2[:, :, :], AF.Exp,
                                         scale=SCALE)
                else:
                    loads["v"] = cv
                    dve_exp(pT[:, :, :], ps2[:, :, :])
            else:
                nc.scalar.activation(pT[:, :, c0:], ps2[:, :, c0:], AF.Exp,
                                     scale=SCALE)
                loads["s"] += cost("s", 2 * (512 - c0))
                # fused triangle multiply over both head-halves
                el = 256
                if loads["g"] + cost("g", el) <= loads["v"] + cost("v", el):
                    loads["g"] += cost("g", el)
                    eng = nc.gpsimd
                else:
                    loads["v"] += cost("v", el)
                    eng = nc.vector
                eng.tensor_tensor(
                    out=pT[:, :, c0:c0 + 128], in0=pT[:, :, c0:c0 + 128],
                    in1=tri_sb[:], op=ALU.mult,
                )
            pTs[n] = pT

        # run-ahead caps: a pre-emitted S slot must never depend (via psum /
        # pT-pool / diag-buffer WAR) on an AV emitted later, or the in-order
        # engine streams deadlock.
        full_alloc = [0]
        full_freed = [0]     # bumped when an AV consuming a pT_full is emitted
        END_OF_IT = [8, 24, 48, 80]

        def allowed(k):
            it, pair, jb = slots[k]
            if jb - 4 * it < 0:           # sub-diagonal: pT_full pool bound
                return full_alloc[0] - full_freed[0] < 16
            # diagonal: double-buffered by it parity; it's buffer was last
            # used by it-2, whose AVs must already be emitted
            return it < 2 or av_n[0] >= END_OF_IT[it - 2]

        def top_up(n):
            while s_cur[0] < min(n, len(slots)) and allowed(s_cur[0]):
                it, pair, jb = slots[s_cur[0]]
                if jb - 4 * it < 0:
                    full_alloc[0] += 1
                emit_S(s_cur[0])
                s_cur[0] += 1

        # ---------------- V projection (packed psum tiles) -------------------
        # V half 1 (tb 0..7, needs xT cols 0:1024 only) runs right after the
        # first quarter-pair, with the it0/it1 S+exp pre-run interleaved; then
        # the second quarter-pair; then V half 2 with the it2 pre-run.
        def v_half(vh, pre):
            for vt in range(2 * vh, 2 * vh + 2):
                vtile = tileCD(f"v{vt}")
                for tb in range(4 * vt, 4 * vt + 4):
                    psv = vtile[:, (tb % 4) // 2,
                                256 * (tb % 2): 256 * (tb % 2) + 256]
                    for o in range(KO):
                        nc.tensor.matmul(
                            psv,
                            lhsT=xT_sb[:, o, 128 * tb: 128 * (tb + 1)],
                            rhs=wv_sb[:, o],
                            start=(o == 0), stop=(o == KO - 1),
                        )
                    psv_r = psv.rearrange("p (h d) -> p h d", h=HPC)
                    drain(V_aug[:, tb, 0::2, 0:64], psv_r[:, 0::2, :], 128)
                    drain(V_aug[:, tb, 1::2, 64:128], psv_r[:, 1::2, :], 128)
                    top_up(pre[tb])

        top_up(8)                    # it0 S+exp (diag tiles only)
        v_half(0, {0: 10, 1: 12, 2: 14, 3: 16, 4: 18, 5: 20, 6: 22, 7: 24})
        qk_pair(1)                   # quarters 2,3
        v_half(1, {8: 27, 9: 30, 10: 33, 11: 36, 12: 39, 13: 42, 14: 45,
                   15: 48})

        # ---------------- AV + finish + interleaved C ----------------
        def emit_AV(it, pair, jb, pT, ctx_t):
            njb = 4 * it + 4
            c0 = max(0, 128 * (jb - 4 * it))
            for hl in range(2):
                h = 2 * pair + hl
                nc.tensor.matmul(
                    ctx_t[:, hl, c0:],
                    lhsT=V_aug[:, jb, h, :],
                    rhs=pT[:, hl, c0:],
                    start=(jb == 0), stop=(jb == njb - 1),
                )

        def finish(it, pair, ctx_t):
            recs = pb.tile([128, 2, 512], FP32, name="recs", tag="recs",
                           bufs=2)
            nc.vector.reciprocal_approx_fast(recs[:], ctx_t[:])
            loads["v"] += cost("v", 1024)
            rcn = pb.tile([128, 512], FP32, name="rec_n", tag="rec_n",
                          bufs=2)
            # issue the half-crossing DMAs from the GpSimd queue (whose
            # multiply consumes them next) so they never head-of-line block
            # the Sync queue's output DMAs.
            nc.gpsimd.dma_start(rcn[0:64, :], recs[64:128, 0, :])
            nc.gpsimd.dma_start(rcn[64:128, :], recs[0:64, 1, :])
            # merged = ctx * rec; hl0 on ScalarE+GpSimd, hl1 on VectorE
            # (straight off PSUM) so the two halves run concurrently.
            m0 = merged[pair][0:64, it]
            nc.scalar.copy(m0, ctx_t[0:64, 0, :])
            nc.gpsimd.tensor_tensor(out=m0, in0=m0, in1=rcn[0:64, :],
                                    op=ALU.mult)
            loads["s"] += cost("s", 512)
            loads["g"] += cost("g", 512)
            nc.vector.tensor_tensor(out=merged[pair][64:128, it],
                                    in0=ctx_t[64:128, 1, :],
                                    in1=rcn[64:128, :], op=ALU.mult)
            loads["v"] += cost("v", 512)

        def emit_C_group(it_, gi):
            tpc = tileAB(f"c{it_}_{gi}")
            for pair in range(NPAIR):
                for e2 in range(2):
                    eb = 2 * gi + e2
                    nc.tensor.matmul(
                        tpc[:, e2],
                        lhsT=wout_sb[:, pair, 128 * eb: 128 * (eb + 1)],
                        rhs=merged[pair][:, it_],
                        start=(pair == 0), stop=(pair == NPAIR - 1),
                    )
            osb = pb.tile([128, 2, 512], BF16, name="osb", tag="osb", bufs=6)
            drain(osb[:], tpc[:], 1024)
            nc.sync.dma_start(
                out_r[:, 2 * gi: 2 * gi + 2, 512 * it_: 512 * (it_ + 1)],
                osb[:],
            )

        L = 14
        cq = []
        for it in range(IT):
            njb = 4 * it + 4
            for pair in range(NPAIR):
                ctx_tag = "CD"[nCD[0] % 2]
                ctx_t = tileCD(f"ctx{it}_{pair}")
                # the vacated CD tag is still draining through the previous
                # pair's finish chain; let it re-enter the S ring only a few
                # slots into this run.
                free_cd[0] = None
                for jb in range(njb):
                    n = av_n[0]
                    if jb < 4 * it:
                        full_freed[0] += 1
                    emit_AV(it, pair, jb, pTs.pop(n), ctx_t)
                    av_n[0] += 1
                    if jb == 2:
                        free_cd[0] = f"ps{'DC'['CD'.index(ctx_tag)]}"
                    top_up(n + 1 + L)
                    # hold back two C groups before the last it so the PE has
                    # ready work to chew on during the final finish chain
                    if cq and jb % 4 == 3 and (it < 3 or len(cq) > 2):
                        emit_C_group(*cq.pop(0))
                finish(it, pair, ctx_t)
                if pair == NPAIR - 1:
                    for gi in range(4):
                        cq.append((it, gi))
        while cq:
            emit_C_group(*cq.pop(0))

    if compile:
        nc.compile()
    return nc


_PROGRAM = None


def _get_program():
    global _PROGRAM
    if _PROGRAM is None:
        _PROGRAM = build_program()
    return _PROGRAM


def _tri():
    dj = np.arange(128)[:, None]
    di = np.arange(128)[None, :]
    t = (dj <= di).astype(ml_dtypes.bfloat16)
    return np.ascontiguousarray(np.concatenate([t, t], axis=1))


def make_in_maps(x, Wqkv, Wout):
    in_maps = []
    for core in range(NCORES):
        b, hg = core // (NCORES // B), core % (NCORES // B)
        c0 = hg * HPC * Dh
        csl = slice(c0, c0 + HPC * Dh)
        wqk_full = np.concatenate(
            [Wqkv[:, csl], Wqkv[:, D + c0: D + c0 + HPC * Dh]], axis=1
        ).astype(ml_dtypes.bfloat16)
        wv_full = Wqkv[:, 2 * D + c0: 2 * D + c0 + HPC * Dh].astype(
            ml_dtypes.bfloat16)
        in_maps.append({
            "tri": _tri(),
            "xT": np.ascontiguousarray(x[b].T).astype(ml_dtypes.bfloat16),
            "wqk": np.ascontiguousarray(
                wqk_full.reshape(KO, 128, 2 * HPC * Dh).transpose(1, 0, 2)),
            "wv": np.ascontiguousarray(
                wv_full.reshape(KO, 128, HPC * Dh).transpose(1, 0, 2)),
            "wout": np.ascontiguousarray(
                Wout[csl, :].astype(ml_dtypes.bfloat16)
                .reshape(2, 128, D).transpose(1, 0, 2)),
        })
    return in_maps


def kernel(x, causal_mask, key_padding_mask, Wqkv, bqkv, Wout, bout,
           _trace=False):
    from concourse.bass_utils import run_bass_kernel_spmd

    x = np.asarray(x, dtype=np.float32)
    Wqkv = np.asarray(Wqkv, dtype=np.float32)
    Wout = np.asarray(Wout, dtype=np.float32)
    bqkv = np.asarray(bqkv, dtype=np.float32)
    bout = np.asarray(bout, dtype=np.float32)
    if np.any(np.asarray(key_padding_mask)):
        raise NotImplementedError("key_padding_mask with padded keys")
    if np.any(bqkv):
        raise NotImplementedError("nonzero bqkv")

    nc = _get_program()
    in_maps = make_in_maps(x, Wqkv, Wout)
    res = run_bass_kernel_spmd(nc, in_maps, core_ids=list(range(NCORES)),
                               trace=_trace)
    G = NCORES // B
    outp = np.empty((B, T, D), dtype=np.float32)
    for b in range(B):
        acc = res.results[b * G]["outT"].astype(np.float32)
        for hg in range(1, G):
            acc += res.results[b * G + hg]["outT"].astype(np.float32)
        outp[b] = acc.T + bout
    kernel.last_exec_time_ns = res.exec_time_ns
    return outp
